# revision 6
# baseline (speedup 1.0000x reference)
"""MoE expert FFN (swiglu) kernel for 8 trn2 NeuronCores.

Expert parallelism: 8 experts, one per core. Each core computes, for its
expert e:
    h   = x_e @ w1_e            # [2048, 2048] @ [2048, 2816]
    act = silu(h[:, :1408]) * h[:, 1408:]
    out = act @ w2_e            # [2048, 1408] @ [1408, 2048]

Tokens arrive pre-sorted by expert with equal counts (2048/expert), so
sharding is a static slice and the gather is a concat. No collectives.

Device-side layout (all bf16 compute, fp32 PSUM accumulation, bf16 out):
  mm1: out[f, t] tiles; lhsT = w1[h,f] 128x128 tiles (stationary),
       rhs = xT[h, t] (moving, N=512) -> inter is [f, t], the layout mm2
       needs, so no on-device transpose anywhere (x is transposed on host).
  swiglu pairs: w1 columns are interleaved on HOST so pair j = cols
       [256j, 256j+256) = [a_j | b_j]; act_j = silu(a)*b via ACT(Silu)
       + DVE mul -> bf16 SBUF.
  mm2: out[t, h] tiles; lhsT = act[f, t] 128-col slices (stationary),
       rhs = w2[f, h] (moving, N=512). PSUM -> SBUF bf16 -> DMA to out.

v6: DMA-native host packing. The v5 trace showed the kernel at the PE
instruction floor (216ns per N=512 matmul) with ~35us of overhead, all
DMA-induced: w1 k-slices were strided reads (512B segments, 5.6KB
stride -> ~25GB/s effective), causing 13us before the first matmul and
12us of PE gaps in the first 65us, plus strided output stores in the
tail. Fixes:
  - w1 is host-packed per swiglu pair as [128 partitions][k][256] so a
    pair's full k-range is ONE contiguous-per-partition DMA (8KB/part).
    Pair 0 is split in 4 granules and pair 1 in 2 so the PE can start
    as soon as the first 256KB lands.
  - output stores one full m-tile row-block [128, 2048] per DMA
    (contiguous 512KB) after packing the 4 n-block PSUM copies into a
    single SBUF tile.
  - queue split: sync HWDGE = all w1 then all w2 (its 4-slot rotation
    gives a 4-transfer prefetch window); gpsimd SWDGE = all x chunks;
    scalar HWDGE = x0's last k slice + output stores (scalar's
    instruction stream also runs silu + PSUM copies, so it must carry
    few DMA configs).
  - 6 warmup matmuls on a zeroed scratch tile fill the pre-first-data
    window so the PE's HAM clock gate (cold = 1.2GHz for the first
    ~3.4us of activity) warms on throwaway work instead of real work.

mm1 is software-staggered over two 512-token chunks (step t runs pair t
of chunk 0 and pair t-2 of chunk 1) per 1024-token super-chunk; weights
are then reused across chunks at half the stream rate. mm2 runs per
super-chunk (8 m-tiles); the very last m-tile is n-outer so its PSUM
banks drain while the PE finishes -> shorter tail.

Weights stay resident in SBUF (bf16: 88KB + 44KB per partition).
PE-bound: ~456us of matmul per core at 2.4GHz; target is wall ~= that.
"""

import os
import sys

sys.path.insert(0, "/opt/trn_rl_repo")

import numpy as np
import ml_dtypes

E = 8             # experts == cores
T_TOTAL = 16384
H = 2048
F = 1408
F2 = 2 * F        # 2816
TPC = T_TOTAL // E  # 2048 tokens per core
CHUNK = 512
NSC = 2                     # super-chunks
NCI = 2                     # chunks per super-chunk
KH = H // 128               # 16 contraction tiles for mm1
NF = F // 128               # 11 swiglu pairs
NT = (NCI * CHUNK) // 128   # 8 m-tiles per super-chunk in mm2
NHO = H // 512              # 4 output column blocks

# DMA granules per w1 pair: pair j is stored [128][k][256] contiguously;
# G[j] transfers cover its KH k-slices. Pair 0 gates the PE start, so it
# is split fine; later pairs arrive a full step ahead at 1MB each.
W1_GRAN = {0: 4, 1: 2}

_CACHE = {}

# Optional knobs read by test.py (not used by the grading harness).
TRACE = os.environ.get("BASS_TRACE_KERNEL", "0") == "1"
LAST = {}


def _build():
    from concourse import bacc, tile, mybir

    bf16 = mybir.dt.bfloat16
    f32 = mybir.dt.float32
    SILU = mybir.ActivationFunctionType.Silu

    # Bacc (not plain Bass): its lowering pipeline splits multi-sem waits
    # into EventSemaphore pairs — TRN2 allows at most 1 wait per instruction.
    nc = bacc.Bacc()
    # x is host-packed as [p, chunk, k, t] -> [128, NCH*KH*CHUNK]: any k-range
    # of one chunk is a single contiguous 2D DMA slice, so x streams in
    # k-PAIR transfers (half the configs of per-k slices).
    xT_d = nc.declare_dram_parameter(
        "xT", [128, (TPC // CHUNK) * KH * CHUNK], bf16, isOutput=False
    )
    # w1 host-packed per partition as [pair][k][256].
    w1_d = nc.declare_dram_parameter("w1", [128, NF * KH * 256], bf16, isOutput=False)
    w2_d = nc.declare_dram_parameter("w2", [F, H], bf16, isOutput=False)
    # bf16 output (host upcasts): halves store bytes + the kernel-tail
    # drain of the final stores. Adds ~0.3% rounding noise on top of the
    # 0.41% bf16-matmul noise — far inside the 2e-2 gate.
    out_d = nc.declare_dram_parameter("out", [TPC, H], bf16, isOutput=True)

    def x_dram_pair(c, kp):
        c0 = (c * KH + 2 * kp) * CHUNK
        return xT_d[:, c0 : c0 + 2 * CHUNK]

    with tile.TileContext(nc) as tc:
        with (
            tc.tile_pool(name="w1p", bufs=1) as w1p,
            tc.tile_pool(name="w2p", bufs=1) as w2p,
            tc.tile_pool(name="xp", bufs=1) as xp,
            tc.tile_pool(name="actp", bufs=1) as actp,
            tc.tile_pool(name="tmpp", bufs=2) as tmpp,
            tc.tile_pool(name="warmp", bufs=1) as warmp,
            tc.tile_pool(name="outp", bufs=2) as outp,
            tc.tile_pool(name="psp", bufs=8, space="PSUM") as psp,
        ):
            # PE warmup: the HAM clock gate holds the PE at 1.2GHz until
            # ~3.4us of sustained activity. Real data lands ~9-10us in
            # (preamble + first transfers), so spend the wait on matmuls
            # over a zeroed scratch tile; the first real matmuls then run
            # at (or much closer to) 2.4GHz. The scratch PSUM tile shares
            # the "ps" tag rotation, so it simply becomes the first of
            # the 8 rotating bank buffers.
            wsrc = warmp.tile([128, 640], bf16, tag="warm")
            nc.vector.memset(wsrc[:], 0.0)
            wps = psp.tile([128, 512], f32, tag="ps", name="warm_ps")
            for _ in range(12):
                nc.tensor.matmul(
                    wps[:], wsrc[:, 0:128], wsrc[:, 128:640], start=True, stop=True
                )

            # x chunk 0 on gpsimd (SWDGE). Irregular k-split
            # [k0][k1,k2]...[k13,k14][k15]: k0 lands first (PE start),
            # and the k15 single rides the scalar queue instead (arrives
            # early vs at the end of gpsimd's stream), so step 0 never
            # waits on its last contraction slice.
            # i-chunk k -> (tile index, col offset) for this split:
            xmap0 = [(0, 0)]
            for k in range(1, KH - 1):
                xmap0.append(((k + 1) // 2, 0) if k % 2 == 1 else (k // 2, CHUNK))
            xmap0.append((KH // 2, 0))

            def x_tiles_i0(c, tag_prefix, name_prefix):
                """Allocate the irregular x tile set for an even chunk c;
                returns (tiles, dma list as (tile, src))."""
                tiles, dmas = [], []
                base = c * KH * CHUNK
                t = xp.tile([128, CHUNK], bf16, tag=f"{tag_prefix}_s0",
                            name=f"{name_prefix}_s0")
                tiles.append(t)
                dmas.append((t, xT_d[:, base : base + CHUNK]))
                for j in range(1, KH // 2):
                    t = xp.tile([128, 2 * CHUNK], bf16, tag=f"{tag_prefix}_p{j}",
                                name=f"{name_prefix}_p{j}")
                    tiles.append(t)
                    c0 = base + (2 * j - 1) * CHUNK
                    dmas.append((t, xT_d[:, c0 : c0 + 2 * CHUNK]))
                t = xp.tile([128, CHUNK], bf16, tag=f"{tag_prefix}_s8",
                            name=f"{name_prefix}_s8")
                tiles.append(t)
                c0 = base + (KH - 1) * CHUNK
                dmas.append((t, xT_d[:, c0 : c0 + CHUNK]))
                return tiles, dmas

            x0_t, x0_dmas = x_tiles_i0(0, "x_0", "x0")
            for t, src in x0_dmas[:-1]:
                nc.gpsimd.dma_start(out=t[:], in_=src)
            # x0's k15 single on scalar: arrives ~early vs the tail of
            # gpsimd's x0 stream.
            nc.scalar.dma_start(out=x0_dmas[-1][0][:], in_=x0_dmas[-1][1])

            # w1 on sync, in consumption order; pair 1 rides the scalar
            # queue so sync's 4-slot rotation prefetches pair 2+ one slot
            # earlier during the startup crunch. Pair j's granule g is the
            # contiguous slice [(j*KH + g*KG)*256, +KG*256) per partition.
            # tiles: w1_t[j][g] of [128, KG*256]; matmul slices columns.
            w1_t = []
            for j in range(NF):
                ng = W1_GRAN.get(j, 1)
                kg = KH // ng
                tiles = []
                for g in range(ng):
                    t = w1p.tile([128, kg * 256], bf16, tag=f"w1_{j}_{g}")
                    tiles.append(t)
                    c0 = (j * KH + g * kg) * 256
                    eng = nc.scalar if j == 1 else nc.sync
                    eng.dma_start(out=t[:], in_=w1_d[:, c0 : c0 + kg * 256])
                w1_t.append(tiles)

            def w1_slice(j, k, half):
                ng = W1_GRAN.get(j, 1)
                kg = KH // ng
                t = w1_t[j][k // kg]
                c = (k % kg) * 256 + half * 128
                return t[:, c : c + 128]

            # x chunk 1 on gpsimd after x0: first used at step 2 (~40us).
            x1_t = []
            for kp in range(KH // 2):
                t = xp.tile([128, 2 * CHUNK], bf16, tag=f"x_1_{kp}", name=f"x1_{kp}")
                x1_t.append(t)
                nc.gpsimd.dma_start(out=t[:], in_=x_dram_pair(1, kp))

            # Resident w2: 11 tiles [128, 2048], each a fully contiguous
            # 512KB DRAM slice; on sync behind w1 (needed ~150us in).
            w2_t = []
            for k in range(NF):
                t = w2p.tile([128, H], bf16, tag=f"w2_{k}")
                w2_t.append(t)
                nc.sync.dma_start(out=t[:], in_=w2_d[k * 128 : (k + 1) * 128, :])

            for S in range(NSC):
                if S == 0:
                    x_t = [x0_t, x1_t]
                else:
                    # Chunks 2+3 reuse chunk 0+1's SBUF slots (released at
                    # the end of super-chunk 0's mm1); the gpsimd queue is
                    # idle by then so the waits cost nothing.
                    c2_t, c2_dmas = x_tiles_i0(S * NCI, "x_0", f"x_{S}_0")
                    for t, src in c2_dmas:
                        nc.gpsimd.dma_start(out=t[:], in_=src)
                    x_t = [c2_t, []]
                    for kp in range(KH // 2):
                        t = xp.tile(
                            [128, 2 * CHUNK], bf16, tag=f"x_1_{kp}",
                            name=f"x_{S}_1_{kp}",
                        )
                        x_t[1].append(t)
                        nc.gpsimd.dma_start(
                            out=t[:], in_=x_dram_pair(S * NCI + 1, kp)
                        )

                # mm1 + swiglu, software-staggered: step t runs pair t of
                # chunk 0 and pair t-2 of chunk 1. Chunk 1's weights are
                # always two pairs old (resident), so only (x0, pair0)
                # are on the DMA critical path at startup.
                STAG = 2
                act_t = [[None] * NF, [None] * NF]  # [chunk][j]
                for t_s in range(NF + STAG):
                    chains = []
                    if t_s < NF:
                        chains.append((0, t_s))
                    if t_s >= STAG:
                        chains.append((1, t_s - STAG))
                    for i, j in chains:
                        ps_a = psp.tile(
                            [128, CHUNK], f32, tag="ps", name=f"ps_{S}_{i}_{j}_a"
                        )
                        ps_b = psp.tile(
                            [128, CHUNK], f32, tag="ps", name=f"ps_{S}_{i}_{j}_b"
                        )
                        for k in range(KH):
                            st, sp = (k == 0), (k == KH - 1)
                            if i == 0:
                                ti, xo = xmap0[k]
                            else:
                                ti, xo = k // 2, (k % 2) * CHUNK
                            xk = x_t[i][ti][:, xo : xo + CHUNK]
                            nc.tensor.matmul(
                                ps_a[:], w1_slice(j, k, 0), xk, start=st, stop=sp
                            )
                            nc.tensor.matmul(
                                ps_b[:], w1_slice(j, k, 1), xk, start=st, stop=sp
                            )
                        tmp = tmpp.tile([128, CHUNK], f32, tag="tmp")
                        nc.scalar.activation(tmp[:], ps_a[:], SILU)
                        a = actp.tile([128, CHUNK], bf16, tag=f"act_{i}_{j}")
                        act_t[i][j] = a
                        nc.vector.tensor_mul(a[:], tmp[:], ps_b[:])

                # mm2: out[t, h], 8 m-tiles per super-chunk. k-outer/
                # n-inner keeps 4 PSUM banks accumulating; the very last
                # m-tile flips to n-outer so each bank finishes early and
                # its copy + store overlap the remaining matmuls. The 4
                # n-block copies pack into ONE [128, 2048] SBUF tile so
                # the store is a single contiguous 512KB DMA.
                for m in range(NT):
                    i, mc = divmod(m, NT // NCI)
                    last = (S == NSC - 1) and (m == NT - 1)
                    po = [
                        psp.tile([128, 512], f32, tag="ps", name=f"po_{S}_{m}_{n}")
                        for n in range(NHO)
                    ]
                    osb = outp.tile([128, H], bf16, tag="osb")
                    r0 = (S * NCI + i) * CHUNK + mc * 128
                    if last:
                        # n-outer, and each n-slice stores right after its
                        # copy (sync queue): when the final matmul retires
                        # only one 128KB store remains -> short tail.
                        for n in range(NHO):
                            for k in range(NF):
                                nc.tensor.matmul(
                                    po[n][:],
                                    act_t[i][k][:, mc * 128 : (mc + 1) * 128],
                                    w2_t[k][:, n * 512 : (n + 1) * 512],
                                    start=(k == 0),
                                    stop=(k == NF - 1),
                                )
                            nc.scalar.copy(
                                osb[:, n * 512 : (n + 1) * 512], po[n][:]
                            )
                            nc.sync.dma_start(
                                out=out_d[r0 : r0 + 128, n * 512 : (n + 1) * 512],
                                in_=osb[:, n * 512 : (n + 1) * 512],
                            )
                    else:
                        for k in range(NF):
                            lhsT = act_t[i][k][:, mc * 128 : (mc + 1) * 128]
                            for n in range(NHO):
                                nc.tensor.matmul(
                                    po[n][:],
                                    lhsT,
                                    w2_t[k][:, n * 512 : (n + 1) * 512],
                                    start=(k == 0),
                                    stop=(k == NF - 1),
                                )
                        for n in range(NHO):
                            nc.scalar.copy(
                                osb[:, n * 512 : (n + 1) * 512], po[n][:]
                            )
                        nc.scalar.dma_start(out=out_d[r0 : r0 + 128, :], in_=osb[:])
    if not nc.is_finalized():
        nc.finalize()  # Bacc.finalize runs the lowering pipeline (sem split, alloc_regs)
    return nc


def _get_nc():
    if "nc" not in _CACHE:
        _CACHE["nc"] = _build()
    return _CACHE["nc"]


def _pack_w1(w1e: np.ndarray) -> np.ndarray:
    """[H, 2816] -> [128, NF*KH*256] DMA-native layout.

    Swiglu pair j = (a_j = cols [128j,128j+128), b_j = cols
    [1408+128j, ...)). Per partition p (row index within a k-tile), the
    packed layout is [pair j][k][a_j|b_j cols], so one pair's whole
    k-range is contiguous per partition.
    """
    a = w1e[:, :F].reshape(KH, 128, NF, 128)   # [k, p, j, c]
    b = w1e[:, F:].reshape(KH, 128, NF, 128)
    pair = np.stack([a, b], axis=3)            # [k, p, j, 2, c]
    # -> [p, j, k, 2*128]
    return pair.transpose(1, 2, 0, 3, 4).reshape(128, NF * KH * 256)


def kernel(permuted_hidden_states, num_tokens_per_expert, w1, w2):
    from concourse.bass_utils import run_bass_kernel_spmd

    x = np.asarray(permuted_hidden_states, dtype=np.float32)
    w1 = np.asarray(w1, dtype=np.float32)
    w2 = np.asarray(w2, dtype=np.float32)
    ntpe = np.asarray(num_tokens_per_expert)
    assert x.shape == (T_TOTAL, H) and w1.shape == (E, H, F2) and w2.shape == (E, F, H)
    # Reference semantics rely on the static equal split.
    assert np.all(ntpe == TPC), f"expected equal {TPC}-token splits, got {ntpe}"

    bf = ml_dtypes.bfloat16
    in_maps = []
    NCH = TPC // CHUNK
    for e in range(E):
        xe = x[e * TPC : (e + 1) * TPC]
        # pack as [p, chunk, k, t] so any k-range of a chunk is one
        # contiguous 2D DMA slice (see xT_d comment in _build).
        xg = (
            xe.reshape(NCH, CHUNK, KH, 128)
            .transpose(3, 0, 2, 1)
            .reshape(128, NCH * KH * CHUNK)
        )
        in_maps.append(
            {
                "xT": np.ascontiguousarray(xg).astype(bf),
                "w1": np.ascontiguousarray(_pack_w1(w1[e])).astype(bf),
                "w2": np.ascontiguousarray(w2[e]).astype(bf),
            }
        )

    nc = _get_nc()
    res = run_bass_kernel_spmd(nc, in_maps, list(range(E)), trace=TRACE)
    LAST["exec_time_ns"] = res.exec_time_ns
    LAST["mean_exec_time_ns"] = res.mean_exec_time_ns
    LAST["profile_json"] = res.profile_json
    out = np.concatenate([res.results[i]["out"] for i in range(E)], axis=0)
    return np.ascontiguousarray(out.astype(np.float32))


# revision 8
# speedup vs baseline: 1.0004x; 1.0004x over previous
"""MoE expert FFN (swiglu) kernel for 8 trn2 NeuronCores.

Expert parallelism: 8 experts, one per core. Each core computes, for its
expert e:
    h   = x_e @ w1_e            # [2048, 2048] @ [2048, 2816]
    act = silu(h[:, :1408]) * h[:, 1408:]
    out = act @ w2_e            # [2048, 1408] @ [1408, 2048]

Tokens arrive pre-sorted by expert with equal counts (2048/expert), so
sharding is a static slice and the gather is a concat. No collectives.

Device-side layout (all bf16 compute, fp32 PSUM accumulation, bf16 out):
  mm1: out[f, t] tiles; lhsT = w1[h,f] 128x128 tiles (stationary),
       rhs = xT[h, t] (moving, N=512) -> inter is [f, t], the layout mm2
       needs, so no on-device transpose anywhere (x is transposed on host).
  swiglu pairs: w1 columns are interleaved on HOST so pair j = cols
       [256j, 256j+256) = [a_j | b_j]; act_j = silu(a)*b via ACT(Silu)
       + DVE mul -> bf16 SBUF.
  mm2: out[t, h] tiles; lhsT = act[f, t] 128-col slices (stationary),
       rhs = w2[f, h] (moving, N=512). PSUM -> SBUF bf16 -> DMA to out.

v6: DMA-native host packing. The v5 trace showed the kernel at the PE
instruction floor (216ns per N=512 matmul) with ~35us of overhead, all
DMA-induced: w1 k-slices were strided reads (512B segments, 5.6KB
stride -> ~25GB/s effective), causing 13us before the first matmul and
12us of PE gaps in the first 65us, plus strided output stores in the
tail. Fixes:
  - w1 is host-packed per swiglu pair as [128 partitions][k][256] so a
    pair's full k-range is ONE contiguous-per-partition DMA (8KB/part).
    Pair 0 is split in 4 granules and pair 1 in 2 so the PE can start
    as soon as the first 256KB lands.
  - output stores one full m-tile row-block [128, 2048] per DMA
    (contiguous 512KB) after packing the 4 n-block PSUM copies into a
    single SBUF tile.
  - queue split: sync HWDGE = all w1 then all w2 (its 4-slot rotation
    gives a 4-transfer prefetch window); gpsimd SWDGE = all x chunks;
    scalar HWDGE = x0's last k slice + output stores (scalar's
    instruction stream also runs silu + PSUM copies, so it must carry
    few DMA configs).
  - 6 warmup matmuls on a zeroed scratch tile fill the pre-first-data
    window so the PE's HAM clock gate (cold = 1.2GHz for the first
    ~3.4us of activity) warms on throwaway work instead of real work.

mm1 is software-staggered over two 512-token chunks (step t runs pair t
of chunk 0 and pair t-2 of chunk 1) per 1024-token super-chunk; weights
are then reused across chunks at half the stream rate. mm2 runs per
super-chunk (8 m-tiles); the very last m-tile is n-outer so its PSUM
banks drain while the PE finishes -> shorter tail.

Weights stay resident in SBUF (bf16: 88KB + 44KB per partition).
PE-bound: ~456us of matmul per core at 2.4GHz; target is wall ~= that.
"""

import os
import sys

sys.path.insert(0, "/opt/trn_rl_repo")

import numpy as np
import ml_dtypes

E = 8             # experts == cores
T_TOTAL = 16384
H = 2048
F = 1408
F2 = 2 * F        # 2816
TPC = T_TOTAL // E  # 2048 tokens per core
CHUNK = 512
NSC = 2                     # super-chunks
NCI = 2                     # chunks per super-chunk
KH = H // 128               # 16 contraction tiles for mm1
NF = F // 128               # 11 swiglu pairs
NT = (NCI * CHUNK) // 128   # 8 m-tiles per super-chunk in mm2
NHO = H // 512              # 4 output column blocks

# DMA granules per w1 pair: pair j is stored [128][k][256] contiguously;
# G[j] transfers cover its KH k-slices. Pair 0 gates the PE start, so it
# is split fine; later pairs arrive a full step ahead at 1MB each.
W1_GRAN = {0: 4, 1: 2}

_CACHE = {}

# Optional knobs read by test.py (not used by the grading harness).
TRACE = os.environ.get("BASS_TRACE_KERNEL", "0") == "1"
LAST = {}


def _build():
    from concourse import bacc, tile, mybir

    bf16 = mybir.dt.bfloat16
    f32 = mybir.dt.float32
    SILU = mybir.ActivationFunctionType.Silu

    # Bacc (not plain Bass): its lowering pipeline splits multi-sem waits
    # into EventSemaphore pairs — TRN2 allows at most 1 wait per instruction.
    nc = bacc.Bacc()
    # x is host-packed as [p, chunk, k, t] -> [128, NCH*KH*CHUNK]: any k-range
    # of one chunk is a single contiguous 2D DMA slice, so x streams in
    # k-PAIR transfers (half the configs of per-k slices).
    xT_d = nc.declare_dram_parameter(
        "xT", [128, (TPC // CHUNK) * KH * CHUNK], bf16, isOutput=False
    )
    # w1 host-packed per partition as [pair][k][256].
    w1_d = nc.declare_dram_parameter("w1", [128, NF * KH * 256], bf16, isOutput=False)
    w2_d = nc.declare_dram_parameter("w2", [F, H], bf16, isOutput=False)
    # bf16 output (host upcasts): halves store bytes + the kernel-tail
    # drain of the final stores. Adds ~0.3% rounding noise on top of the
    # 0.41% bf16-matmul noise — far inside the 2e-2 gate.
    out_d = nc.declare_dram_parameter("out", [TPC, H], bf16, isOutput=True)

    def x_dram_pair(c, kp):
        c0 = (c * KH + 2 * kp) * CHUNK
        return xT_d[:, c0 : c0 + 2 * CHUNK]

    with tile.TileContext(nc) as tc:
        with (
            tc.tile_pool(name="w1p", bufs=1) as w1p,
            tc.tile_pool(name="w2p", bufs=1) as w2p,
            tc.tile_pool(name="xp", bufs=1) as xp,
            tc.tile_pool(name="actp", bufs=1) as actp,
            tc.tile_pool(name="tmpp", bufs=2) as tmpp,
            tc.tile_pool(name="warmp", bufs=1) as warmp,
            tc.tile_pool(name="outp", bufs=2) as outp,
            tc.tile_pool(name="psp", bufs=8, space="PSUM") as psp,
        ):
            # PE warmup: the HAM clock gate holds the PE at 1.2GHz until
            # ~3.4us of sustained activity. Real data lands ~9-10us in
            # (preamble + first transfers), so spend the wait on matmuls
            # over a zeroed scratch tile; the first real matmuls then run
            # at (or much closer to) 2.4GHz. The scratch PSUM tile shares
            # the "ps" tag rotation, so it simply becomes the first of
            # the 8 rotating bank buffers.
            wsrc = warmp.tile([128, 640], bf16, tag="warm")
            nc.vector.memset(wsrc[:], 0.0)
            wps = psp.tile([128, 512], f32, tag="ps", name="warm_ps")
            for _ in range(12):
                nc.tensor.matmul(
                    wps[:], wsrc[:, 0:128], wsrc[:, 128:640], start=True, stop=True
                )

            # x chunk 0 on gpsimd (SWDGE). Irregular k-split
            # [k0][k1,k2]...[k13,k14][k15]: k0 lands first (PE start),
            # and the k15 single rides the scalar queue instead (arrives
            # early vs at the end of gpsimd's stream), so step 0 never
            # waits on its last contraction slice.
            # i-chunk k -> (tile index, col offset) for this split:
            xmap0 = [(0, 0)]
            for k in range(1, KH - 1):
                xmap0.append(((k + 1) // 2, 0) if k % 2 == 1 else (k // 2, CHUNK))
            xmap0.append((KH // 2, 0))

            def x_tiles_i0(c, tag_prefix, name_prefix):
                """Allocate the irregular x tile set for an even chunk c;
                returns (tiles, dma list as (tile, src))."""
                tiles, dmas = [], []
                base = c * KH * CHUNK
                t = xp.tile([128, CHUNK], bf16, tag=f"{tag_prefix}_s0",
                            name=f"{name_prefix}_s0")
                tiles.append(t)
                dmas.append((t, xT_d[:, base : base + CHUNK]))
                for j in range(1, KH // 2):
                    t = xp.tile([128, 2 * CHUNK], bf16, tag=f"{tag_prefix}_p{j}",
                                name=f"{name_prefix}_p{j}")
                    tiles.append(t)
                    c0 = base + (2 * j - 1) * CHUNK
                    dmas.append((t, xT_d[:, c0 : c0 + 2 * CHUNK]))
                t = xp.tile([128, CHUNK], bf16, tag=f"{tag_prefix}_s8",
                            name=f"{name_prefix}_s8")
                tiles.append(t)
                c0 = base + (KH - 1) * CHUNK
                dmas.append((t, xT_d[:, c0 : c0 + CHUNK]))
                return tiles, dmas

            x0_t, x0_dmas = x_tiles_i0(0, "x_0", "x0")
            for t, src in x0_dmas[:-1]:
                nc.gpsimd.dma_start(out=t[:], in_=src)
            # x0's k15 single on scalar: arrives ~early vs the tail of
            # gpsimd's x0 stream.
            nc.scalar.dma_start(out=x0_dmas[-1][0][:], in_=x0_dmas[-1][1])

            # w1 on sync, in consumption order; pair 1 rides the scalar
            # queue so sync's 4-slot rotation prefetches pair 2+ one slot
            # earlier during the startup crunch. Pair j's granule g is the
            # contiguous slice [(j*KH + g*KG)*256, +KG*256) per partition.
            # tiles: w1_t[j][g] of [128, KG*256]; matmul slices columns.
            w1_t = []
            for j in range(NF):
                ng = W1_GRAN.get(j, 1)
                kg = KH // ng
                tiles = []
                for g in range(ng):
                    t = w1p.tile([128, kg * 256], bf16, tag=f"w1_{j}_{g}")
                    tiles.append(t)
                    c0 = (j * KH + g * kg) * 256
                    eng = nc.scalar if j == 1 else nc.sync
                    eng.dma_start(out=t[:], in_=w1_d[:, c0 : c0 + kg * 256])
                w1_t.append(tiles)

            def w1_slice(j, k, half):
                ng = W1_GRAN.get(j, 1)
                kg = KH // ng
                t = w1_t[j][k // kg]
                c = (k % kg) * 256 + half * 128
                return t[:, c : c + 128]

            # x chunk 1 on gpsimd after x0: first used at step 2 (~40us).
            x1_t = []
            for kp in range(KH // 2):
                t = xp.tile([128, 2 * CHUNK], bf16, tag=f"x_1_{kp}", name=f"x1_{kp}")
                x1_t.append(t)
                nc.gpsimd.dma_start(out=t[:], in_=x_dram_pair(1, kp))

            # Resident w2: 11 tiles [128, 2048], each a fully contiguous
            # 512KB DRAM slice; on sync behind w1 (needed ~150us in).
            w2_t = []
            for k in range(NF):
                t = w2p.tile([128, H], bf16, tag=f"w2_{k}")
                w2_t.append(t)
                nc.sync.dma_start(out=t[:], in_=w2_d[k * 128 : (k + 1) * 128, :])

            for S in range(NSC):
                if S == 0:
                    x_t = [x0_t, x1_t]
                else:
                    # Chunks 2+3 reuse chunk 0+1's SBUF slots (released at
                    # the end of super-chunk 0's mm1); the gpsimd queue is
                    # idle by then so the waits cost nothing.
                    c2_t, c2_dmas = x_tiles_i0(S * NCI, "x_0", f"x_{S}_0")
                    for t, src in c2_dmas:
                        nc.gpsimd.dma_start(out=t[:], in_=src)
                    x_t = [c2_t, []]
                    for kp in range(KH // 2):
                        t = xp.tile(
                            [128, 2 * CHUNK], bf16, tag=f"x_1_{kp}",
                            name=f"x_{S}_1_{kp}",
                        )
                        x_t[1].append(t)
                        nc.gpsimd.dma_start(
                            out=t[:], in_=x_dram_pair(S * NCI + 1, kp)
                        )

                # mm1 + swiglu, software-staggered: step t runs pair t of
                # chunk 0 and pair t-4 of chunk 1. Chunk 1's weights are
                # always four pairs old (resident), so only (x0, pair0)
                # are on the DMA critical path at startup, and x1's
                # deadline (step STAG, ~41us) sits past the startup HBM
                # crunch: peak required delivery rate drops from ~360GB/s
                # (infeasible under 8-core HBM contention) to ~226GB/s.
                # Chain count and PE work are unchanged by the stagger.
                STAG = 4
                act_t = [[None] * NF, [None] * NF]  # [chunk][j]
                for t_s in range(NF + STAG):
                    chains = []
                    if t_s < NF:
                        chains.append((0, t_s))
                    if t_s >= STAG:
                        chains.append((1, t_s - STAG))
                    for i, j in chains:
                        ps_a = psp.tile(
                            [128, CHUNK], f32, tag="ps", name=f"ps_{S}_{i}_{j}_a"
                        )
                        ps_b = psp.tile(
                            [128, CHUNK], f32, tag="ps", name=f"ps_{S}_{i}_{j}_b"
                        )
                        for k in range(KH):
                            st, sp = (k == 0), (k == KH - 1)
                            if i == 0:
                                ti, xo = xmap0[k]
                            else:
                                ti, xo = k // 2, (k % 2) * CHUNK
                            xk = x_t[i][ti][:, xo : xo + CHUNK]
                            nc.tensor.matmul(
                                ps_a[:], w1_slice(j, k, 0), xk, start=st, stop=sp
                            )
                            nc.tensor.matmul(
                                ps_b[:], w1_slice(j, k, 1), xk, start=st, stop=sp
                            )
                        tmp = tmpp.tile([128, CHUNK], f32, tag="tmp")
                        nc.scalar.activation(tmp[:], ps_a[:], SILU)
                        a = actp.tile([128, CHUNK], bf16, tag=f"act_{i}_{j}")
                        act_t[i][j] = a
                        nc.vector.tensor_mul(a[:], tmp[:], ps_b[:])

                # mm2: out[t, h], 8 m-tiles per super-chunk. k-outer/
                # n-inner keeps 4 PSUM banks accumulating; the very last
                # m-tile flips to n-outer so each bank finishes early and
                # its copy + store overlap the remaining matmuls. The 4
                # n-block copies pack into ONE [128, 2048] SBUF tile so
                # the store is a single contiguous 512KB DMA.
                for m in range(NT):
                    i, mc = divmod(m, NT // NCI)
                    last = (S == NSC - 1) and (m == NT - 1)
                    po = [
                        psp.tile([128, 512], f32, tag="ps", name=f"po_{S}_{m}_{n}")
                        for n in range(NHO)
                    ]
                    osb = outp.tile([128, H], bf16, tag="osb")
                    r0 = (S * NCI + i) * CHUNK + mc * 128
                    if last:
                        # n-outer, and each n-slice stores right after its
                        # copy (sync queue): when the final matmul retires
                        # only one 64KB store remains -> short tail. The
                        # final 512-col block runs as two 256-col chains
                        # so the very last copy/store is half-size.
                        pieces = [(n * 512, 512) for n in range(NHO - 1)]
                        pieces += [(1536, 256), (1792, 256)]
                        for c0, cw in pieces:
                            for k in range(NF):
                                nc.tensor.matmul(
                                    po[c0 // 512][:, c0 % 512 : c0 % 512 + cw],
                                    act_t[i][k][:, mc * 128 : (mc + 1) * 128],
                                    w2_t[k][:, c0 : c0 + cw],
                                    start=(k == 0),
                                    stop=(k == NF - 1),
                                )
                            nc.scalar.copy(
                                osb[:, c0 : c0 + cw],
                                po[c0 // 512][:, c0 % 512 : c0 % 512 + cw],
                            )
                            nc.sync.dma_start(
                                out=out_d[r0 : r0 + 128, c0 : c0 + cw],
                                in_=osb[:, c0 : c0 + cw],
                            )
                    else:
                        for k in range(NF):
                            lhsT = act_t[i][k][:, mc * 128 : (mc + 1) * 128]
                            for n in range(NHO):
                                nc.tensor.matmul(
                                    po[n][:],
                                    lhsT,
                                    w2_t[k][:, n * 512 : (n + 1) * 512],
                                    start=(k == 0),
                                    stop=(k == NF - 1),
                                )
                        for n in range(NHO):
                            nc.scalar.copy(
                                osb[:, n * 512 : (n + 1) * 512], po[n][:]
                            )
                        nc.scalar.dma_start(out=out_d[r0 : r0 + 128, :], in_=osb[:])
    if not nc.is_finalized():
        nc.finalize()  # Bacc.finalize runs the lowering pipeline (sem split, alloc_regs)
    return nc


def _get_nc():
    if "nc" not in _CACHE:
        _CACHE["nc"] = _build()
    return _CACHE["nc"]


def _pack_w1(w1e: np.ndarray) -> np.ndarray:
    """[H, 2816] -> [128, NF*KH*256] DMA-native layout.

    Swiglu pair j = (a_j = cols [128j,128j+128), b_j = cols
    [1408+128j, ...)). Per partition p (row index within a k-tile), the
    packed layout is [pair j][k][a_j|b_j cols], so one pair's whole
    k-range is contiguous per partition.
    """
    a = w1e[:, :F].reshape(KH, 128, NF, 128)   # [k, p, j, c]
    b = w1e[:, F:].reshape(KH, 128, NF, 128)
    pair = np.stack([a, b], axis=3)            # [k, p, j, 2, c]
    # -> [p, j, k, 2*128]
    return pair.transpose(1, 2, 0, 3, 4).reshape(128, NF * KH * 256)


def kernel(permuted_hidden_states, num_tokens_per_expert, w1, w2):
    from concourse.bass_utils import run_bass_kernel_spmd

    x = np.asarray(permuted_hidden_states, dtype=np.float32)
    w1 = np.asarray(w1, dtype=np.float32)
    w2 = np.asarray(w2, dtype=np.float32)
    ntpe = np.asarray(num_tokens_per_expert)
    assert x.shape == (T_TOTAL, H) and w1.shape == (E, H, F2) and w2.shape == (E, F, H)
    # Reference semantics rely on the static equal split.
    assert np.all(ntpe == TPC), f"expected equal {TPC}-token splits, got {ntpe}"

    bf = ml_dtypes.bfloat16
    in_maps = []
    NCH = TPC // CHUNK
    for e in range(E):
        xe = x[e * TPC : (e + 1) * TPC]
        # pack as [p, chunk, k, t] so any k-range of a chunk is one
        # contiguous 2D DMA slice (see xT_d comment in _build).
        xg = (
            xe.reshape(NCH, CHUNK, KH, 128)
            .transpose(3, 0, 2, 1)
            .reshape(128, NCH * KH * CHUNK)
        )
        in_maps.append(
            {
                "xT": np.ascontiguousarray(xg).astype(bf),
                "w1": np.ascontiguousarray(_pack_w1(w1[e])).astype(bf),
                "w2": np.ascontiguousarray(w2[e]).astype(bf),
            }
        )

    nc = _get_nc()
    res = run_bass_kernel_spmd(nc, in_maps, list(range(E)), trace=TRACE)
    LAST["exec_time_ns"] = res.exec_time_ns
    LAST["mean_exec_time_ns"] = res.mean_exec_time_ns
    LAST["profile_json"] = res.profile_json
    out = np.concatenate([res.results[i]["out"] for i in range(E)], axis=0)
    return np.ascontiguousarray(out.astype(np.float32))


# revision 12
# speedup vs baseline: 1.0138x; 1.0133x over previous
"""MoE expert FFN (swiglu) kernel for 8 trn2 NeuronCores.

Expert parallelism: 8 experts, one per core. Each core computes, for its
expert e:
    h   = x_e @ w1_e            # [2048, 2048] @ [2048, 2816]
    act = silu(h[:, :1408]) * h[:, 1408:]
    out = act @ w2_e            # [2048, 1408] @ [1408, 2048]

Tokens arrive pre-sorted by expert with equal counts (2048/expert), so
sharding is a static slice and the gather is a concat. No collectives.

Device-side layout (all bf16 compute, fp32 PSUM accumulation, bf16 out):
  mm1: out[f, t] tiles; lhsT = w1[h,f] 128x128 tiles (stationary),
       rhs = xT[h, t] (moving, N=512) -> inter is [f, t], the layout mm2
       needs, so no on-device transpose anywhere (x is transposed on host).
  swiglu pairs: w1 columns are interleaved on HOST so pair j = cols
       [256j, 256j+256) = [a_j | b_j]; act_j = silu(a)*b via ACT(Silu)
       + DVE mul -> bf16 SBUF.
  mm2: out[t, h] tiles; lhsT = act[f, t] 128-col slices (stationary),
       rhs = w2[f, h] (moving, N=512). PSUM -> SBUF bf16 -> DMA to out.

v6: DMA-native host packing. The v5 trace showed the kernel at the PE
instruction floor (216ns per N=512 matmul) with ~35us of overhead, all
DMA-induced: w1 k-slices were strided reads (512B segments, 5.6KB
stride -> ~25GB/s effective), causing 13us before the first matmul and
12us of PE gaps in the first 65us, plus strided output stores in the
tail. Fixes:
  - w1 is host-packed per swiglu pair as [128 partitions][k][256] so a
    pair's full k-range is ONE contiguous-per-partition DMA (8KB/part).
    Pair 0 is split in 4 granules and pair 1 in 2 so the PE can start
    as soon as the first 256KB lands.
  - output stores one full m-tile row-block [128, 2048] per DMA
    (contiguous 512KB) after packing the 4 n-block PSUM copies into a
    single SBUF tile.
  - queue split: sync HWDGE = all w1 then all w2 (its 4-slot rotation
    gives a 4-transfer prefetch window); gpsimd SWDGE = all x chunks;
    scalar HWDGE = x0's last k slice + output stores (scalar's
    instruction stream also runs silu + PSUM copies, so it must carry
    few DMA configs).
  - 6 warmup matmuls on a zeroed scratch tile fill the pre-first-data
    window so the PE's HAM clock gate (cold = 1.2GHz for the first
    ~3.4us of activity) warms on throwaway work instead of real work.

mm1 is software-staggered over two 512-token chunks (step t runs pair t
of chunk 0 and pair t-2 of chunk 1) per 1024-token super-chunk; weights
are then reused across chunks at half the stream rate. mm2 runs per
super-chunk (8 m-tiles); the very last m-tile is n-outer so its PSUM
banks drain while the PE finishes -> shorter tail.

Weights stay resident in SBUF (bf16: 88KB + 44KB per partition).
PE-bound: ~456us of matmul per core at 2.4GHz; target is wall ~= that.
"""

import os
import sys

sys.path.insert(0, "/opt/trn_rl_repo")

import numpy as np
import ml_dtypes

E = 8             # experts == cores
T_TOTAL = 16384
H = 2048
F = 1408
F2 = 2 * F        # 2816
TPC = T_TOTAL // E  # 2048 tokens per core
CHUNK = 512
NSC = 2                     # super-chunks
NCI = 2                     # chunks per super-chunk
KH = H // 128               # 16 contraction tiles for mm1
NF = F // 128               # 11 swiglu pairs
NT = (NCI * CHUNK) // 128   # 8 m-tiles per super-chunk in mm2
NHO = H // 512              # 4 output column blocks

# DMA granules per w1 pair: pair j is stored [128][k][256] contiguously;
# G[j] transfers cover its KH k-slices. Pair 0 gates the PE start, so it
# is split fine; later pairs arrive a full step ahead at 1MB each.
W1_GRAN = {0: 4, 1: 2}

_CACHE = {}

# Optional knobs read by test.py (not used by the grading harness).
TRACE = os.environ.get("BASS_TRACE_KERNEL", "0") == "1"
LAST = {}


def _build():
    from concourse import bacc, tile, mybir

    bf16 = mybir.dt.bfloat16
    f32 = mybir.dt.float32
    SILU = mybir.ActivationFunctionType.Silu

    # Bacc (not plain Bass): its lowering pipeline splits multi-sem waits
    # into EventSemaphore pairs — TRN2 allows at most 1 wait per instruction.
    nc = bacc.Bacc()
    # x is host-packed as [p, chunk, k, t] -> [128, NCH*KH*CHUNK]: any k-range
    # of one chunk is a single contiguous 2D DMA slice, so x streams in
    # k-PAIR transfers (half the configs of per-k slices).
    xT_d = nc.declare_dram_parameter(
        "xT", [128, (TPC // CHUNK) * KH * CHUNK], bf16, isOutput=False
    )
    # w1 host-packed per partition as [pair][k][256].
    w1_d = nc.declare_dram_parameter("w1", [128, NF * KH * 256], bf16, isOutput=False)
    w2_d = nc.declare_dram_parameter("w2", [F, H], bf16, isOutput=False)
    # bf16 output (host upcasts): halves store bytes + the kernel-tail
    # drain of the final stores. Adds ~0.3% rounding noise on top of the
    # 0.41% bf16-matmul noise — far inside the 2e-2 gate.
    out_d = nc.declare_dram_parameter("out", [TPC, H], bf16, isOutput=True)

    def x_dram_pair(c, kp):
        c0 = (c * KH + 2 * kp) * CHUNK
        return xT_d[:, c0 : c0 + 2 * CHUNK]

    with tile.TileContext(nc) as tc:
        with (
            tc.tile_pool(name="w1p", bufs=1) as w1p,
            tc.tile_pool(name="w2p", bufs=1) as w2p,
            tc.tile_pool(name="xp", bufs=1) as xp,
            tc.tile_pool(name="actp", bufs=1) as actp,
            tc.tile_pool(name="tmpp", bufs=2) as tmpp,
            tc.tile_pool(name="warmp", bufs=1) as warmp,
            tc.tile_pool(name="outp", bufs=2) as outp,
            tc.tile_pool(name="psp", bufs=8, space="PSUM") as psp,
        ):
            # PE warmup: the HAM clock gate holds the PE at 1.2GHz until
            # ~3.4us of sustained activity. Real data lands ~9-10us in
            # (preamble + first transfers), so spend the wait on matmuls
            # over a zeroed scratch tile; the first real matmuls then run
            # at (or much closer to) 2.4GHz. The scratch PSUM tile shares
            # the "ps" tag rotation, so it simply becomes the first of
            # the 8 rotating bank buffers.
            wsrc = warmp.tile([128, 640], bf16, tag="warm")
            nc.vector.memset(wsrc[:], 0.0)
            wps = psp.tile([128, 512], f32, tag="ps", name="warm_ps")
            for _ in range(6):
                nc.tensor.matmul(
                    wps[:], wsrc[:, 0:128], wsrc[:, 128:640], start=True, stop=True
                )

            # x chunk 0 on gpsimd (SWDGE). Irregular k-split
            # [k0][k1,k2]...[k13,k14][k15]: k0 lands first (PE start),
            # and the k15 single rides the scalar queue instead (arrives
            # early vs at the end of gpsimd's stream), so step 0 never
            # waits on its last contraction slice.
            # i-chunk k -> (tile index, col offset) for this split:
            xmap0 = [(0, 0)]
            for k in range(1, KH - 1):
                xmap0.append(((k + 1) // 2, 0) if k % 2 == 1 else (k // 2, CHUNK))
            xmap0.append((KH // 2, 0))

            def x_tiles_i0(c, tag_prefix, name_prefix):
                """Allocate the irregular x tile set for an even chunk c;
                returns (tiles, dma list as (tile, src))."""
                tiles, dmas = [], []
                base = c * KH * CHUNK
                t = xp.tile([128, CHUNK], bf16, tag=f"{tag_prefix}_s0",
                            name=f"{name_prefix}_s0")
                tiles.append(t)
                dmas.append((t, xT_d[:, base : base + CHUNK]))
                for j in range(1, KH // 2):
                    t = xp.tile([128, 2 * CHUNK], bf16, tag=f"{tag_prefix}_p{j}",
                                name=f"{name_prefix}_p{j}")
                    tiles.append(t)
                    c0 = base + (2 * j - 1) * CHUNK
                    dmas.append((t, xT_d[:, c0 : c0 + 2 * CHUNK]))
                t = xp.tile([128, CHUNK], bf16, tag=f"{tag_prefix}_s8",
                            name=f"{name_prefix}_s8")
                tiles.append(t)
                c0 = base + (KH - 1) * CHUNK
                dmas.append((t, xT_d[:, c0 : c0 + CHUNK]))
                return tiles, dmas

            # x chunk 0 is THE startup-critical transfer set (2.2MB the
            # first chain consumes over ~7us): split it across the gpsimd
            # SWDGE queue (k0-6) and the scalar HWDGE queue (k7-15) so it
            # arrives in roughly half the time of a single queue.
            x0_t, x0_dmas = x_tiles_i0(0, "x_0", "x0")
            for t, src in x0_dmas[:4]:
                nc.gpsimd.dma_start(out=t[:], in_=src)
            for t, src in x0_dmas[4:]:
                nc.scalar.dma_start(out=t[:], in_=src)

            # w1 on sync, in consumption order. Pair j's granule g is the
            # contiguous slice [(j*KH + g*KG)*256, +KG*256) per partition.
            # tiles: w1_t[j][g] of [128, KG*256]; matmul slices columns.
            w1_t = []
            for j in range(NF):
                ng = W1_GRAN.get(j, 1)
                kg = KH // ng
                tiles = []
                for g in range(ng):
                    t = w1p.tile([128, kg * 256], bf16, tag=f"w1_{j}_{g}")
                    tiles.append(t)
                    c0 = (j * KH + g * kg) * 256
                    nc.sync.dma_start(out=t[:], in_=w1_d[:, c0 : c0 + kg * 256])
                w1_t.append(tiles)

            def w1_slice(j, k, half):
                ng = W1_GRAN.get(j, 1)
                kg = KH // ng
                t = w1_t[j][k // kg]
                c = (k % kg) * 256 + half * 128
                return t[:, c : c + 128]

            # x chunk 1 on gpsimd after x0: first used at step 2 (~40us).
            x1_t = []
            for kp in range(KH // 2):
                t = xp.tile([128, 2 * CHUNK], bf16, tag=f"x_1_{kp}", name=f"x1_{kp}")
                x1_t.append(t)
                nc.gpsimd.dma_start(out=t[:], in_=x_dram_pair(1, kp))

            # Resident w2: 11 tiles [128, 2048], each a fully contiguous
            # 512KB DRAM slice; on sync behind w1 (needed ~150us in).
            w2_t = []
            for k in range(NF):
                t = w2p.tile([128, H], bf16, tag=f"w2_{k}")
                w2_t.append(t)
                nc.sync.dma_start(out=t[:], in_=w2_d[k * 128 : (k + 1) * 128, :])

            for S in range(NSC):
                if S == 0:
                    x_t = [x0_t, x1_t]
                else:
                    # Chunks 2+3 reuse chunk 0+1's SBUF slots (released at
                    # the end of super-chunk 0's mm1); the gpsimd queue is
                    # idle by then so the waits cost nothing.
                    c2_t, c2_dmas = x_tiles_i0(S * NCI, "x_0", f"x_{S}_0")
                    for t, src in c2_dmas:
                        nc.gpsimd.dma_start(out=t[:], in_=src)
                    x_t = [c2_t, []]
                    for kp in range(KH // 2):
                        t = xp.tile(
                            [128, 2 * CHUNK], bf16, tag=f"x_1_{kp}",
                            name=f"x_{S}_1_{kp}",
                        )
                        x_t[1].append(t)
                        nc.gpsimd.dma_start(
                            out=t[:], in_=x_dram_pair(S * NCI + 1, kp)
                        )

                # mm1 + swiglu, software-staggered: step t runs pair t of
                # chunk 0 and pair t-2 of chunk 1. Chunk 1's weights are
                # always two pairs old (resident), so only (x0, pair0)
                # are on the DMA critical path at startup. (STAG=4 was
                # tried and regressed: single-chain prefix steps consume
                # w1 pairs at double rate, so the weight deadlines move
                # earlier by exactly what x1's deadline gains.)
                STAG = 2
                act_t = [[None] * NF, [None] * NF]  # [chunk][j]
                for t_s in range(NF + STAG):
                    chains = []
                    if t_s < NF:
                        chains.append((0, t_s))
                    if t_s >= STAG:
                        chains.append((1, t_s - STAG))
                    for i, j in chains:
                        ps_a = psp.tile(
                            [128, CHUNK], f32, tag="ps", name=f"ps_{S}_{i}_{j}_a"
                        )
                        ps_b = psp.tile(
                            [128, CHUNK], f32, tag="ps", name=f"ps_{S}_{i}_{j}_b"
                        )
                        for k in range(KH):
                            st, sp = (k == 0), (k == KH - 1)
                            if i == 0:
                                ti, xo = xmap0[k]
                            else:
                                ti, xo = k // 2, (k % 2) * CHUNK
                            xk = x_t[i][ti][:, xo : xo + CHUNK]
                            nc.tensor.matmul(
                                ps_a[:], w1_slice(j, k, 0), xk, start=st, stop=sp
                            )
                            nc.tensor.matmul(
                                ps_b[:], w1_slice(j, k, 1), xk, start=st, stop=sp
                            )
                        tmp = tmpp.tile([128, CHUNK], f32, tag="tmp")
                        nc.scalar.activation(tmp[:], ps_a[:], SILU)
                        a = actp.tile([128, CHUNK], bf16, tag=f"act_{i}_{j}")
                        act_t[i][j] = a
                        nc.vector.tensor_mul(a[:], tmp[:], ps_b[:])

                # mm2: out[t, h], 8 m-tiles per super-chunk. k-outer/
                # n-inner keeps 4 PSUM banks accumulating; the very last
                # m-tile flips to n-outer so each bank finishes early and
                # its copy + store overlap the remaining matmuls. The 4
                # n-block copies pack into ONE [128, 2048] SBUF tile so
                # the store is a single contiguous 512KB DMA.
                for m in range(NT):
                    i, mc = divmod(m, NT // NCI)
                    last = (S == NSC - 1) and (m == NT - 1)
                    po = [
                        psp.tile([128, 512], f32, tag="ps", name=f"po_{S}_{m}_{n}")
                        for n in range(NHO)
                    ]
                    osb = outp.tile([128, H], bf16, tag="osb")
                    r0 = (S * NCI + i) * CHUNK + mc * 128
                    if last:
                        # n-outer, and each n-slice stores right after its
                        # copy (sync queue): when the final matmul retires
                        # only one 64KB store remains -> short tail. The
                        # final 512-col block runs as two 256-col chains
                        # so the very last copy/store is half-size.
                        pieces = [(n * 512, 512) for n in range(NHO - 1)]
                        pieces += [(1536, 256), (1792, 256)]
                        for c0, cw in pieces:
                            for k in range(NF):
                                nc.tensor.matmul(
                                    po[c0 // 512][:, c0 % 512 : c0 % 512 + cw],
                                    act_t[i][k][:, mc * 128 : (mc + 1) * 128],
                                    w2_t[k][:, c0 : c0 + cw],
                                    start=(k == 0),
                                    stop=(k == NF - 1),
                                )
                            nc.scalar.copy(
                                osb[:, c0 : c0 + cw],
                                po[c0 // 512][:, c0 % 512 : c0 % 512 + cw],
                            )
                            nc.sync.dma_start(
                                out=out_d[r0 : r0 + 128, c0 : c0 + cw],
                                in_=osb[:, c0 : c0 + cw],
                            )
                    else:
                        for k in range(NF):
                            lhsT = act_t[i][k][:, mc * 128 : (mc + 1) * 128]
                            for n in range(NHO):
                                nc.tensor.matmul(
                                    po[n][:],
                                    lhsT,
                                    w2_t[k][:, n * 512 : (n + 1) * 512],
                                    start=(k == 0),
                                    stop=(k == NF - 1),
                                )
                        for n in range(NHO):
                            nc.scalar.copy(
                                osb[:, n * 512 : (n + 1) * 512], po[n][:]
                            )
                        nc.scalar.dma_start(out=out_d[r0 : r0 + 128, :], in_=osb[:])
    if not nc.is_finalized():
        nc.finalize()  # Bacc.finalize runs the lowering pipeline (sem split, alloc_regs)
    return nc


def _get_nc():
    if "nc" not in _CACHE:
        _CACHE["nc"] = _build()
    return _CACHE["nc"]


def _pack_w1(w1e: np.ndarray) -> np.ndarray:
    """[H, 2816] -> [128, NF*KH*256] DMA-native layout.

    Swiglu pair j = (a_j = cols [128j,128j+128), b_j = cols
    [1408+128j, ...)). Per partition p (row index within a k-tile), the
    packed layout is [pair j][k][a_j|b_j cols], so one pair's whole
    k-range is contiguous per partition.
    """
    a = w1e[:, :F].reshape(KH, 128, NF, 128)   # [k, p, j, c]
    b = w1e[:, F:].reshape(KH, 128, NF, 128)
    pair = np.stack([a, b], axis=3)            # [k, p, j, 2, c]
    # -> [p, j, k, 2*128]
    return pair.transpose(1, 2, 0, 3, 4).reshape(128, NF * KH * 256)


def kernel(permuted_hidden_states, num_tokens_per_expert, w1, w2):
    from concourse.bass_utils import run_bass_kernel_spmd

    x = np.asarray(permuted_hidden_states, dtype=np.float32)
    w1 = np.asarray(w1, dtype=np.float32)
    w2 = np.asarray(w2, dtype=np.float32)
    ntpe = np.asarray(num_tokens_per_expert)
    assert x.shape == (T_TOTAL, H) and w1.shape == (E, H, F2) and w2.shape == (E, F, H)
    # Reference semantics rely on the static equal split.
    assert np.all(ntpe == TPC), f"expected equal {TPC}-token splits, got {ntpe}"

    bf = ml_dtypes.bfloat16
    in_maps = []
    NCH = TPC // CHUNK
    for e in range(E):
        xe = x[e * TPC : (e + 1) * TPC]
        # pack as [p, chunk, k, t] so any k-range of a chunk is one
        # contiguous 2D DMA slice (see xT_d comment in _build).
        xg = (
            xe.reshape(NCH, CHUNK, KH, 128)
            .transpose(3, 0, 2, 1)
            .reshape(128, NCH * KH * CHUNK)
        )
        in_maps.append(
            {
                "xT": np.ascontiguousarray(xg).astype(bf),
                "w1": np.ascontiguousarray(_pack_w1(w1[e])).astype(bf),
                "w2": np.ascontiguousarray(w2[e]).astype(bf),
            }
        )

    nc = _get_nc()
    res = run_bass_kernel_spmd(nc, in_maps, list(range(E)), trace=TRACE)
    LAST["exec_time_ns"] = res.exec_time_ns
    LAST["mean_exec_time_ns"] = res.mean_exec_time_ns
    LAST["profile_json"] = res.profile_json
    out = np.concatenate([res.results[i]["out"] for i in range(E)], axis=0)
    return np.ascontiguousarray(out.astype(np.float32))


# revision 14
# speedup vs baseline: 1.0184x; 1.0045x over previous
"""MoE expert FFN (swiglu) kernel for 8 trn2 NeuronCores.

Expert parallelism: 8 experts, one per core. Each core computes, for its
expert e:
    h   = x_e @ w1_e            # [2048, 2048] @ [2048, 2816]
    act = silu(h[:, :1408]) * h[:, 1408:]
    out = act @ w2_e            # [2048, 1408] @ [1408, 2048]

Tokens arrive pre-sorted by expert with equal counts (2048/expert), so
sharding is a static slice and the gather is a concat. No collectives.

Device-side layout (all bf16 compute, fp32 PSUM accumulation, bf16 out):
  mm1: out[f, t] tiles; lhsT = w1[h,f] 128x128 tiles (stationary),
       rhs = xT[h, t] (moving, N=512) -> inter is [f, t], the layout mm2
       needs, so no on-device transpose anywhere (x is transposed on host).
  swiglu pairs: w1 columns are interleaved on HOST so pair j = cols
       [256j, 256j+256) = [a_j | b_j]; act_j = silu(a)*b via ACT(Silu)
       + DVE mul -> bf16 SBUF.
  mm2: out[t, h] tiles; lhsT = act[f, t] 128-col slices (stationary),
       rhs = w2[f, h] (moving, N=512). PSUM -> SBUF bf16 -> DMA to out.

v6: DMA-native host packing. The v5 trace showed the kernel at the PE
instruction floor (216ns per N=512 matmul) with ~35us of overhead, all
DMA-induced: w1 k-slices were strided reads (512B segments, 5.6KB
stride -> ~25GB/s effective), causing 13us before the first matmul and
12us of PE gaps in the first 65us, plus strided output stores in the
tail. Fixes:
  - w1 is host-packed per swiglu pair as [128 partitions][k][256] so a
    pair's full k-range is ONE contiguous-per-partition DMA (8KB/part).
    Pair 0 is split in 4 granules and pair 1 in 2 so the PE can start
    as soon as the first 256KB lands.
  - output stores one full m-tile row-block [128, 2048] per DMA
    (contiguous 512KB) after packing the 4 n-block PSUM copies into a
    single SBUF tile.
  - queue split: sync HWDGE = all w1 then all w2 (its 4-slot rotation
    gives a 4-transfer prefetch window); gpsimd SWDGE = all x chunks;
    scalar HWDGE = x0's last k slice + output stores (scalar's
    instruction stream also runs silu + PSUM copies, so it must carry
    few DMA configs).
  - 6 warmup matmuls on a zeroed scratch tile fill the pre-first-data
    window so the PE's HAM clock gate (cold = 1.2GHz for the first
    ~3.4us of activity) warms on throwaway work instead of real work.

mm1 is software-staggered over two 512-token chunks (step t runs pair t
of chunk 0 and pair t-2 of chunk 1) per 1024-token super-chunk; weights
are then reused across chunks at half the stream rate. mm2 runs per
super-chunk (8 m-tiles); the very last m-tile is n-outer so its PSUM
banks drain while the PE finishes -> shorter tail.

Weights stay resident in SBUF (bf16: 88KB + 44KB per partition).
PE-bound: ~456us of matmul per core at 2.4GHz; target is wall ~= that.
"""

import os
import sys

sys.path.insert(0, "/opt/trn_rl_repo")

import numpy as np
import ml_dtypes

E = 8             # experts == cores
T_TOTAL = 16384
H = 2048
F = 1408
F2 = 2 * F        # 2816
TPC = T_TOTAL // E  # 2048 tokens per core
CHUNK = 512
NSC = 2                     # super-chunks
NCI = 2                     # chunks per super-chunk
KH = H // 128               # 16 contraction tiles for mm1
NF = F // 128               # 11 swiglu pairs
NT = (NCI * CHUNK) // 128   # 8 m-tiles per super-chunk in mm2
NHO = H // 512              # 4 output column blocks

# DMA granules per w1 pair: pair j is stored [128][k][256] contiguously;
# G[j] transfers cover its KH k-slices. Pair 0 gates the PE start, so it
# is split fine; later pairs arrive a full step ahead at 1MB each.
W1_GRAN = {0: 4, 1: 2}

_CACHE = {}

# Optional knobs read by test.py (not used by the grading harness).
TRACE = os.environ.get("BASS_TRACE_KERNEL", "0") == "1"
LAST = {}


def _build():
    from concourse import bacc, tile, mybir

    bf16 = mybir.dt.bfloat16
    f32 = mybir.dt.float32
    SILU = mybir.ActivationFunctionType.Silu

    # Bacc (not plain Bass): its lowering pipeline splits multi-sem waits
    # into EventSemaphore pairs — TRN2 allows at most 1 wait per instruction.
    nc = bacc.Bacc()
    # x is host-packed as [p, chunk, k, t] -> [128, NCH*KH*CHUNK]: any k-range
    # of one chunk is a single contiguous 2D DMA slice, so x streams in
    # k-PAIR transfers (half the configs of per-k slices).
    xT_d = nc.declare_dram_parameter(
        "xT", [128, (TPC // CHUNK) * KH * CHUNK], bf16, isOutput=False
    )
    # w1 host-packed per partition as [pair][k][256].
    w1_d = nc.declare_dram_parameter("w1", [128, NF * KH * 256], bf16, isOutput=False)
    w2_d = nc.declare_dram_parameter("w2", [F, H], bf16, isOutput=False)
    # bf16 output (host upcasts): halves store bytes + the kernel-tail
    # drain of the final stores. Adds ~0.3% rounding noise on top of the
    # 0.41% bf16-matmul noise — far inside the 2e-2 gate.
    out_d = nc.declare_dram_parameter("out", [TPC, H], bf16, isOutput=True)

    def x_dram_pair(c, kp):
        c0 = (c * KH + 2 * kp) * CHUNK
        return xT_d[:, c0 : c0 + 2 * CHUNK]

    with tile.TileContext(nc) as tc:
        with (
            tc.tile_pool(name="w1p", bufs=1) as w1p,
            tc.tile_pool(name="w2p", bufs=1) as w2p,
            tc.tile_pool(name="xp", bufs=1) as xp,
            tc.tile_pool(name="actp", bufs=1) as actp,
            tc.tile_pool(name="tmpp", bufs=2) as tmpp,
            tc.tile_pool(name="warmp", bufs=1) as warmp,
            tc.tile_pool(name="outp", bufs=2) as outp,
            tc.tile_pool(name="psp", bufs=8, space="PSUM") as psp,
        ):
            # PE warmup: the HAM clock gate holds the PE at 1.2GHz until
            # ~3.4us of sustained activity. Real data lands ~9-10us in
            # (preamble + first transfers), so spend the wait on matmuls
            # over a zeroed scratch tile; the first real matmuls then run
            # at (or much closer to) 2.4GHz. The scratch PSUM tile shares
            # the "ps" tag rotation, so it simply becomes the first of
            # the 8 rotating bank buffers.
            wsrc = warmp.tile([128, 640], bf16, tag="warm")
            nc.vector.memset(wsrc[:], 0.0)
            wps = psp.tile([128, 512], f32, tag="ps", name="warm_ps")
            for _ in range(6):
                nc.tensor.matmul(
                    wps[:], wsrc[:, 0:128], wsrc[:, 128:640], start=True, stop=True
                )

            # x chunk 0 on gpsimd (SWDGE). Irregular k-split
            # [k0][k1,k2]...[k13,k14][k15]: k0 lands first (PE start),
            # and the k15 single rides the scalar queue instead (arrives
            # early vs at the end of gpsimd's stream), so step 0 never
            # waits on its last contraction slice.
            # i-chunk k -> (tile index, col offset) for this split:
            xmap0 = [(0, 0)]
            for k in range(1, KH - 1):
                xmap0.append(((k + 1) // 2, 0) if k % 2 == 1 else (k // 2, CHUNK))
            xmap0.append((KH // 2, 0))

            def x_tiles_i0(c, tag_prefix, name_prefix):
                """Allocate the irregular x tile set for an even chunk c;
                returns (tiles, dma list as (tile, src))."""
                tiles, dmas = [], []
                base = c * KH * CHUNK
                t = xp.tile([128, CHUNK], bf16, tag=f"{tag_prefix}_s0",
                            name=f"{name_prefix}_s0")
                tiles.append(t)
                dmas.append((t, xT_d[:, base : base + CHUNK]))
                for j in range(1, KH // 2):
                    t = xp.tile([128, 2 * CHUNK], bf16, tag=f"{tag_prefix}_p{j}",
                                name=f"{name_prefix}_p{j}")
                    tiles.append(t)
                    c0 = base + (2 * j - 1) * CHUNK
                    dmas.append((t, xT_d[:, c0 : c0 + 2 * CHUNK]))
                t = xp.tile([128, CHUNK], bf16, tag=f"{tag_prefix}_s8",
                            name=f"{name_prefix}_s8")
                tiles.append(t)
                c0 = base + (KH - 1) * CHUNK
                dmas.append((t, xT_d[:, c0 : c0 + CHUNK]))
                return tiles, dmas

            # x chunk 0 is THE startup-critical transfer set (2.2MB the
            # first chain consumes over ~7us): split it across the gpsimd
            # SWDGE queue (k0-6) and the scalar HWDGE queue (k7-15) so it
            # arrives in roughly half the time of a single queue.
            x0_t, x0_dmas = x_tiles_i0(0, "x_0", "x0")
            for t, src in x0_dmas[:4]:
                nc.gpsimd.dma_start(out=t[:], in_=src)
            for t, src in x0_dmas[4:]:
                nc.scalar.dma_start(out=t[:], in_=src)

            # w1 on sync, in consumption order. Pair j's granule g is the
            # contiguous slice [(j*KH + g*KG)*256, +KG*256) per partition.
            # tiles: w1_t[j][g] of [128, KG*256]; matmul slices columns.
            w1_t = []
            for j in range(NF):
                ng = W1_GRAN.get(j, 1)
                kg = KH // ng
                tiles = []
                for g in range(ng):
                    t = w1p.tile([128, kg * 256], bf16, tag=f"w1_{j}_{g}")
                    tiles.append(t)
                    c0 = (j * KH + g * kg) * 256
                    nc.sync.dma_start(out=t[:], in_=w1_d[:, c0 : c0 + kg * 256])
                w1_t.append(tiles)

            def w1_slice(j, k, half):
                ng = W1_GRAN.get(j, 1)
                kg = KH // ng
                t = w1_t[j][k // kg]
                c = (k % kg) * 256 + half * 128
                return t[:, c : c + 128]

            # x chunk 1 on gpsimd after x0: first used at step 2 (~40us).
            x1_t = []
            for kp in range(KH // 2):
                t = xp.tile([128, 2 * CHUNK], bf16, tag=f"x_1_{kp}", name=f"x1_{kp}")
                x1_t.append(t)
                nc.gpsimd.dma_start(out=t[:], in_=x_dram_pair(1, kp))

            # Resident w2: 11 tiles [128, 2048], each a fully contiguous
            # 512KB DRAM slice; on sync behind w1 (needed ~150us in).
            w2_t = []
            for k in range(NF):
                t = w2p.tile([128, H], bf16, tag=f"w2_{k}")
                w2_t.append(t)
                nc.sync.dma_start(out=t[:], in_=w2_d[k * 128 : (k + 1) * 128, :])

            for S in range(NSC):
                if S == 0:
                    x_t = [x0_t, x1_t]
                else:
                    # Chunks 2+3 reuse chunk 0+1's SBUF slots (released at
                    # the end of super-chunk 0's mm1); the gpsimd queue is
                    # idle by then so the waits cost nothing.
                    c2_t, c2_dmas = x_tiles_i0(S * NCI, "x_0", f"x_{S}_0")
                    for t, src in c2_dmas:
                        nc.gpsimd.dma_start(out=t[:], in_=src)
                    x_t = [c2_t, []]
                    for kp in range(KH // 2):
                        t = xp.tile(
                            [128, 2 * CHUNK], bf16, tag=f"x_1_{kp}",
                            name=f"x_{S}_1_{kp}",
                        )
                        x_t[1].append(t)
                        nc.gpsimd.dma_start(
                            out=t[:], in_=x_dram_pair(S * NCI + 1, kp)
                        )

                # mm1 + swiglu, software-staggered: step t runs pair t of
                # chunk 0 and pair t-2 of chunk 1. Chunk 1's weights are
                # always two pairs old (resident), so only (x0, pair0)
                # are on the DMA critical path at startup. (STAG=4 was
                # tried and regressed: single-chain prefix steps consume
                # w1 pairs at double rate, so the weight deadlines move
                # earlier by exactly what x1's deadline gains.)
                STAG = 2
                act_t = [[None] * NF, [None] * NF]  # [chunk][j]
                for t_s in range(NF + STAG):
                    # Chunk-1's chain runs FIRST within a step: its weights
                    # (pair t-2) are already resident, so the fresh pair t
                    # gets an extra chain (~7us) of DMA slack before the
                    # chunk-0 chain needs it.
                    chains = []
                    if t_s >= STAG:
                        chains.append((1, t_s - STAG))
                    if t_s < NF:
                        chains.append((0, t_s))
                    for i, j in chains:
                        ps_a = psp.tile(
                            [128, CHUNK], f32, tag="ps", name=f"ps_{S}_{i}_{j}_a"
                        )
                        ps_b = psp.tile(
                            [128, CHUNK], f32, tag="ps", name=f"ps_{S}_{i}_{j}_b"
                        )
                        for k in range(KH):
                            st, sp = (k == 0), (k == KH - 1)
                            if i == 0:
                                ti, xo = xmap0[k]
                            else:
                                ti, xo = k // 2, (k % 2) * CHUNK
                            xk = x_t[i][ti][:, xo : xo + CHUNK]
                            nc.tensor.matmul(
                                ps_a[:], w1_slice(j, k, 0), xk, start=st, stop=sp
                            )
                            nc.tensor.matmul(
                                ps_b[:], w1_slice(j, k, 1), xk, start=st, stop=sp
                            )
                        tmp = tmpp.tile([128, CHUNK], f32, tag="tmp")
                        nc.scalar.activation(tmp[:], ps_a[:], SILU)
                        a = actp.tile([128, CHUNK], bf16, tag=f"act_{i}_{j}")
                        act_t[i][j] = a
                        nc.vector.tensor_mul(a[:], tmp[:], ps_b[:])

                # mm2: out[t, h], 8 m-tiles per super-chunk. k-outer/
                # n-inner keeps 4 PSUM banks accumulating; the very last
                # m-tile flips to n-outer so each bank finishes early and
                # its copy + store overlap the remaining matmuls. The 4
                # n-block copies pack into ONE [128, 2048] SBUF tile so
                # the store is a single contiguous 512KB DMA.
                for m in range(NT):
                    i, mc = divmod(m, NT // NCI)
                    last = (S == NSC - 1) and (m == NT - 1)
                    osb = outp.tile([128, H], bf16, tag="osb")
                    r0 = (S * NCI + i) * CHUNK + mc * 128
                    if last:
                        # n-outer, and each n-slice stores right after its
                        # copy (sync queue): when the final matmul retires
                        # only one 64KB store remains -> short tail. The
                        # final 512-col block runs as two 256-col chains
                        # so the very last copy/store is half-size. Each
                        # piece gets its OWN psum tile: sharing one tile
                        # between the two 256 pieces made Tile serialize
                        # piece 2's chain behind piece 1's copy.
                        pieces = [(n * 512, 512) for n in range(NHO - 1)]
                        pieces += [(1536, 256), (1792, 256)]
                        for c0, cw in pieces:
                            pp = psp.tile(
                                [128, cw], f32, tag="ps", name=f"po_{S}_{m}_{c0}"
                            )
                            for k in range(NF):
                                nc.tensor.matmul(
                                    pp[:],
                                    act_t[i][k][:, mc * 128 : (mc + 1) * 128],
                                    w2_t[k][:, c0 : c0 + cw],
                                    start=(k == 0),
                                    stop=(k == NF - 1),
                                )
                            nc.scalar.copy(osb[:, c0 : c0 + cw], pp[:])
                            nc.sync.dma_start(
                                out=out_d[r0 : r0 + 128, c0 : c0 + cw],
                                in_=osb[:, c0 : c0 + cw],
                            )
                    else:
                        po = [
                            psp.tile(
                                [128, 512], f32, tag="ps", name=f"po_{S}_{m}_{n}"
                            )
                            for n in range(NHO)
                        ]
                        for k in range(NF):
                            lhsT = act_t[i][k][:, mc * 128 : (mc + 1) * 128]
                            for n in range(NHO):
                                nc.tensor.matmul(
                                    po[n][:],
                                    lhsT,
                                    w2_t[k][:, n * 512 : (n + 1) * 512],
                                    start=(k == 0),
                                    stop=(k == NF - 1),
                                )
                        for n in range(NHO):
                            nc.scalar.copy(
                                osb[:, n * 512 : (n + 1) * 512], po[n][:]
                            )
                        nc.scalar.dma_start(out=out_d[r0 : r0 + 128, :], in_=osb[:])
    if not nc.is_finalized():
        nc.finalize()  # Bacc.finalize runs the lowering pipeline (sem split, alloc_regs)
    return nc


def _get_nc():
    if "nc" not in _CACHE:
        _CACHE["nc"] = _build()
    return _CACHE["nc"]


def _pack_w1(w1e: np.ndarray) -> np.ndarray:
    """[H, 2816] -> [128, NF*KH*256] DMA-native layout.

    Swiglu pair j = (a_j = cols [128j,128j+128), b_j = cols
    [1408+128j, ...)). Per partition p (row index within a k-tile), the
    packed layout is [pair j][k][a_j|b_j cols], so one pair's whole
    k-range is contiguous per partition.
    """
    a = w1e[:, :F].reshape(KH, 128, NF, 128)   # [k, p, j, c]
    b = w1e[:, F:].reshape(KH, 128, NF, 128)
    pair = np.stack([a, b], axis=3)            # [k, p, j, 2, c]
    # -> [p, j, k, 2*128]
    return pair.transpose(1, 2, 0, 3, 4).reshape(128, NF * KH * 256)


def kernel(permuted_hidden_states, num_tokens_per_expert, w1, w2):
    from concourse.bass_utils import run_bass_kernel_spmd

    x = np.asarray(permuted_hidden_states, dtype=np.float32)
    w1 = np.asarray(w1, dtype=np.float32)
    w2 = np.asarray(w2, dtype=np.float32)
    ntpe = np.asarray(num_tokens_per_expert)
    assert x.shape == (T_TOTAL, H) and w1.shape == (E, H, F2) and w2.shape == (E, F, H)
    # Reference semantics rely on the static equal split.
    assert np.all(ntpe == TPC), f"expected equal {TPC}-token splits, got {ntpe}"

    bf = ml_dtypes.bfloat16
    in_maps = []
    NCH = TPC // CHUNK
    for e in range(E):
        xe = x[e * TPC : (e + 1) * TPC]
        # pack as [p, chunk, k, t] so any k-range of a chunk is one
        # contiguous 2D DMA slice (see xT_d comment in _build).
        xg = (
            xe.reshape(NCH, CHUNK, KH, 128)
            .transpose(3, 0, 2, 1)
            .reshape(128, NCH * KH * CHUNK)
        )
        in_maps.append(
            {
                "xT": np.ascontiguousarray(xg).astype(bf),
                "w1": np.ascontiguousarray(_pack_w1(w1[e])).astype(bf),
                "w2": np.ascontiguousarray(w2[e]).astype(bf),
            }
        )

    nc = _get_nc()
    res = run_bass_kernel_spmd(nc, in_maps, list(range(E)), trace=TRACE)
    LAST["exec_time_ns"] = res.exec_time_ns
    LAST["mean_exec_time_ns"] = res.mean_exec_time_ns
    LAST["profile_json"] = res.profile_json
    out = np.concatenate([res.results[i]["out"] for i in range(E)], axis=0)
    return np.ascontiguousarray(out.astype(np.float32))


# revision 16
# speedup vs baseline: 1.0193x; 1.0009x over previous
"""MoE expert FFN (swiglu) kernel for 8 trn2 NeuronCores.

Expert parallelism: 8 experts, one per core. Each core computes, for its
expert e:
    h   = x_e @ w1_e            # [2048, 2048] @ [2048, 2816]
    act = silu(h[:, :1408]) * h[:, 1408:]
    out = act @ w2_e            # [2048, 1408] @ [1408, 2048]

Tokens arrive pre-sorted by expert with equal counts (2048/expert), so
sharding is a static slice and the gather is a concat. No collectives.

Device-side layout (all bf16 compute, fp32 PSUM accumulation, bf16 out):
  mm1: out[f, t] tiles; lhsT = w1[h,f] 128x128 tiles (stationary),
       rhs = xT[h, t] (moving, N=512) -> inter is [f, t], the layout mm2
       needs, so no on-device transpose anywhere (x is transposed on host).
  swiglu pairs: w1 columns are interleaved on HOST so pair j = cols
       [256j, 256j+256) = [a_j | b_j]; act_j = silu(a)*b via ACT(Silu)
       + DVE mul -> bf16 SBUF.
  mm2: out[t, h] tiles; lhsT = act[f, t] 128-col slices (stationary),
       rhs = w2[f, h] (moving, N=512). PSUM -> SBUF bf16 -> DMA to out.

v6: DMA-native host packing. The v5 trace showed the kernel at the PE
instruction floor (216ns per N=512 matmul) with ~35us of overhead, all
DMA-induced: w1 k-slices were strided reads (512B segments, 5.6KB
stride -> ~25GB/s effective), causing 13us before the first matmul and
12us of PE gaps in the first 65us, plus strided output stores in the
tail. Fixes:
  - w1 is host-packed per swiglu pair as [128 partitions][k][256] so a
    pair's full k-range is ONE contiguous-per-partition DMA (8KB/part).
    Pair 0 is split in 4 granules and pair 1 in 2 so the PE can start
    as soon as the first 256KB lands.
  - output stores one full m-tile row-block [128, 2048] per DMA
    (contiguous 512KB) after packing the 4 n-block PSUM copies into a
    single SBUF tile.
  - queue split: sync HWDGE = all w1 then all w2 (its 4-slot rotation
    gives a 4-transfer prefetch window); gpsimd SWDGE = all x chunks;
    scalar HWDGE = x0's last k slice + output stores (scalar's
    instruction stream also runs silu + PSUM copies, so it must carry
    few DMA configs).
  - 6 warmup matmuls on a zeroed scratch tile fill the pre-first-data
    window so the PE's HAM clock gate (cold = 1.2GHz for the first
    ~3.4us of activity) warms on throwaway work instead of real work.

mm1 is software-staggered over two 512-token chunks (step t runs pair t
of chunk 0 and pair t-2 of chunk 1) per 1024-token super-chunk; weights
are then reused across chunks at half the stream rate. mm2 runs per
super-chunk (8 m-tiles); the very last m-tile is n-outer so its PSUM
banks drain while the PE finishes -> shorter tail.

Weights stay resident in SBUF (bf16: 88KB + 44KB per partition).
PE-bound: ~456us of matmul per core at 2.4GHz; target is wall ~= that.
"""

import os
import sys

sys.path.insert(0, "/opt/trn_rl_repo")

import numpy as np
import ml_dtypes

E = 8             # experts == cores
T_TOTAL = 16384
H = 2048
F = 1408
F2 = 2 * F        # 2816
TPC = T_TOTAL // E  # 2048 tokens per core
CHUNK = 512
NSC = 2                     # super-chunks
NCI = 2                     # chunks per super-chunk
KH = H // 128               # 16 contraction tiles for mm1
NF = F // 128               # 11 swiglu pairs
NT = (NCI * CHUNK) // 128   # 8 m-tiles per super-chunk in mm2
NHO = H // 512              # 4 output column blocks

# DMA granules per w1 pair: pair j is stored [128][k][256] contiguously;
# G[j] transfers cover its KH k-slices. Pair 0 gates the PE start, so it
# is split fine; later pairs arrive a full step ahead at 1MB each.
W1_GRAN = {0: 4, 1: 2}

_CACHE = {}

# Optional knobs read by test.py (not used by the grading harness).
TRACE = os.environ.get("BASS_TRACE_KERNEL", "0") == "1"
LAST = {}


def _build():
    from concourse import bacc, tile, mybir

    bf16 = mybir.dt.bfloat16
    f32 = mybir.dt.float32
    SILU = mybir.ActivationFunctionType.Silu

    # Bacc (not plain Bass): its lowering pipeline splits multi-sem waits
    # into EventSemaphore pairs — TRN2 allows at most 1 wait per instruction.
    nc = bacc.Bacc()
    # x is host-packed as [p, chunk, k, t] -> [128, NCH*KH*CHUNK]: any k-range
    # of one chunk is a single contiguous 2D DMA slice, so x streams in
    # k-PAIR transfers (half the configs of per-k slices).
    xT_d = nc.declare_dram_parameter(
        "xT", [128, (TPC // CHUNK) * KH * CHUNK], bf16, isOutput=False
    )
    # w1 host-packed per partition as [pair][k][256].
    w1_d = nc.declare_dram_parameter("w1", [128, NF * KH * 256], bf16, isOutput=False)
    w2_d = nc.declare_dram_parameter("w2", [F, H], bf16, isOutput=False)
    # bf16 output (host upcasts): halves store bytes + the kernel-tail
    # drain of the final stores. Adds ~0.3% rounding noise on top of the
    # 0.41% bf16-matmul noise — far inside the 2e-2 gate.
    out_d = nc.declare_dram_parameter("out", [TPC, H], bf16, isOutput=True)

    def x_dram_pair(c, kp):
        c0 = (c * KH + 2 * kp) * CHUNK
        return xT_d[:, c0 : c0 + 2 * CHUNK]

    with tile.TileContext(nc) as tc:
        with (
            tc.tile_pool(name="w1p", bufs=1) as w1p,
            tc.tile_pool(name="w2p", bufs=1) as w2p,
            tc.tile_pool(name="xp", bufs=1) as xp,
            tc.tile_pool(name="actp", bufs=1) as actp,
            tc.tile_pool(name="tmpp", bufs=2) as tmpp,
            tc.tile_pool(name="warmp", bufs=1) as warmp,
            tc.tile_pool(name="outp", bufs=2) as outp,
            tc.tile_pool(name="psp", bufs=8, space="PSUM") as psp,
        ):
            # PE warmup: the HAM clock gate holds the PE at 1.2GHz until
            # ~3.4us of sustained activity. Real data lands ~9-10us in
            # (preamble + first transfers), so spend the wait on matmuls
            # over a zeroed scratch tile; the first real matmuls then run
            # at (or much closer to) 2.4GHz. The scratch PSUM tile shares
            # the "ps" tag rotation, so it simply becomes the first of
            # the 8 rotating bank buffers.
            wsrc = warmp.tile([128, 640], bf16, tag="warm")
            nc.vector.memset(wsrc[:], 0.0)
            wps = psp.tile([128, 512], f32, tag="ps", name="warm_ps")
            for _ in range(9):
                nc.tensor.matmul(
                    wps[:], wsrc[:, 0:128], wsrc[:, 128:640], start=True, stop=True
                )

            # x chunk 0 on gpsimd (SWDGE). Irregular k-split
            # [k0][k1,k2]...[k13,k14][k15]: k0 lands first (PE start),
            # and the k15 single rides the scalar queue instead (arrives
            # early vs at the end of gpsimd's stream), so step 0 never
            # waits on its last contraction slice.
            # i-chunk k -> (tile index, col offset) for this split:
            xmap0 = [(0, 0)]
            for k in range(1, KH - 1):
                xmap0.append(((k + 1) // 2, 0) if k % 2 == 1 else (k // 2, CHUNK))
            xmap0.append((KH // 2, 0))

            def x_tiles_i0(c, tag_prefix, name_prefix):
                """Allocate the irregular x tile set for an even chunk c;
                returns (tiles, dma list as (tile, src))."""
                tiles, dmas = [], []
                base = c * KH * CHUNK
                t = xp.tile([128, CHUNK], bf16, tag=f"{tag_prefix}_s0",
                            name=f"{name_prefix}_s0")
                tiles.append(t)
                dmas.append((t, xT_d[:, base : base + CHUNK]))
                for j in range(1, KH // 2):
                    t = xp.tile([128, 2 * CHUNK], bf16, tag=f"{tag_prefix}_p{j}",
                                name=f"{name_prefix}_p{j}")
                    tiles.append(t)
                    c0 = base + (2 * j - 1) * CHUNK
                    dmas.append((t, xT_d[:, c0 : c0 + 2 * CHUNK]))
                t = xp.tile([128, CHUNK], bf16, tag=f"{tag_prefix}_s8",
                            name=f"{name_prefix}_s8")
                tiles.append(t)
                c0 = base + (KH - 1) * CHUNK
                dmas.append((t, xT_d[:, c0 : c0 + CHUNK]))
                return tiles, dmas

            # x chunk 0 is THE startup-critical transfer set (2.2MB the
            # first chain consumes over ~7us): split it across the gpsimd
            # SWDGE queue (k0-6) and the scalar HWDGE queue (k7-15) so it
            # arrives in roughly half the time of a single queue.
            x0_t, x0_dmas = x_tiles_i0(0, "x_0", "x0")
            for t, src in x0_dmas[:4]:
                nc.gpsimd.dma_start(out=t[:], in_=src)
            for t, src in x0_dmas[4:]:
                nc.scalar.dma_start(out=t[:], in_=src)

            # w1 on sync, in consumption order. Pair j's granule g is the
            # contiguous slice [(j*KH + g*KG)*256, +KG*256) per partition.
            # tiles: w1_t[j][g] of [128, KG*256]; matmul slices columns.
            w1_t = []
            for j in range(NF):
                ng = W1_GRAN.get(j, 1)
                kg = KH // ng
                tiles = []
                for g in range(ng):
                    t = w1p.tile([128, kg * 256], bf16, tag=f"w1_{j}_{g}")
                    tiles.append(t)
                    c0 = (j * KH + g * kg) * 256
                    nc.sync.dma_start(out=t[:], in_=w1_d[:, c0 : c0 + kg * 256])
                w1_t.append(tiles)

            def w1_slice(j, k, half):
                ng = W1_GRAN.get(j, 1)
                kg = KH // ng
                t = w1_t[j][k // kg]
                c = (k % kg) * 256 + half * 128
                return t[:, c : c + 128]

            # x chunk 1 split across gpsimd (k0-7) + scalar (k8-15), each
            # behind its x0 half: first used at step 2 (~22us), and the
            # chunk-1 chain now runs first within its step.
            x1_t = []
            for kp in range(KH // 2):
                t = xp.tile([128, 2 * CHUNK], bf16, tag=f"x_1_{kp}", name=f"x1_{kp}")
                x1_t.append(t)
                eng = nc.gpsimd if kp < KH // 4 else nc.scalar
                eng.dma_start(out=t[:], in_=x_dram_pair(1, kp))

            # Resident w2: 11 tiles [128, 2048], each a fully contiguous
            # 512KB DRAM slice; on sync behind w1 (needed ~150us in).
            w2_t = []
            for k in range(NF):
                t = w2p.tile([128, H], bf16, tag=f"w2_{k}")
                w2_t.append(t)
                nc.sync.dma_start(out=t[:], in_=w2_d[k * 128 : (k + 1) * 128, :])

            for S in range(NSC):
                if S == 0:
                    x_t = [x0_t, x1_t]
                else:
                    # Chunks 2+3 reuse chunk 0+1's SBUF slots (released at
                    # the end of super-chunk 0's mm1); the gpsimd queue is
                    # idle by then so the waits cost nothing.
                    c2_t, c2_dmas = x_tiles_i0(S * NCI, "x_0", f"x_{S}_0")
                    for t, src in c2_dmas:
                        nc.gpsimd.dma_start(out=t[:], in_=src)
                    x_t = [c2_t, []]
                    for kp in range(KH // 2):
                        t = xp.tile(
                            [128, 2 * CHUNK], bf16, tag=f"x_1_{kp}",
                            name=f"x_{S}_1_{kp}",
                        )
                        x_t[1].append(t)
                        nc.gpsimd.dma_start(
                            out=t[:], in_=x_dram_pair(S * NCI + 1, kp)
                        )

                # mm1 + swiglu, software-staggered: step t runs pair t of
                # chunk 0 and pair t-2 of chunk 1. Chunk 1's weights are
                # always two pairs old (resident), so only (x0, pair0)
                # are on the DMA critical path at startup. (STAG=4 was
                # tried and regressed: single-chain prefix steps consume
                # w1 pairs at double rate, so the weight deadlines move
                # earlier by exactly what x1's deadline gains.)
                STAG = 2
                act_t = [[None] * NF, [None] * NF]  # [chunk][j]
                for t_s in range(NF + STAG):
                    # Chunk-1's chain runs FIRST within a step: its weights
                    # (pair t-2) are already resident, so the fresh pair t
                    # gets an extra chain (~7us) of DMA slack before the
                    # chunk-0 chain needs it.
                    chains = []
                    if t_s >= STAG:
                        chains.append((1, t_s - STAG))
                    if t_s < NF:
                        chains.append((0, t_s))
                    for i, j in chains:
                        ps_a = psp.tile(
                            [128, CHUNK], f32, tag="ps", name=f"ps_{S}_{i}_{j}_a"
                        )
                        ps_b = psp.tile(
                            [128, CHUNK], f32, tag="ps", name=f"ps_{S}_{i}_{j}_b"
                        )
                        for k in range(KH):
                            st, sp = (k == 0), (k == KH - 1)
                            if i == 0:
                                ti, xo = xmap0[k]
                            else:
                                ti, xo = k // 2, (k % 2) * CHUNK
                            xk = x_t[i][ti][:, xo : xo + CHUNK]
                            nc.tensor.matmul(
                                ps_a[:], w1_slice(j, k, 0), xk, start=st, stop=sp
                            )
                            nc.tensor.matmul(
                                ps_b[:], w1_slice(j, k, 1), xk, start=st, stop=sp
                            )
                        tmp = tmpp.tile([128, CHUNK], f32, tag="tmp")
                        nc.scalar.activation(tmp[:], ps_a[:], SILU)
                        a = actp.tile([128, CHUNK], bf16, tag=f"act_{i}_{j}")
                        act_t[i][j] = a
                        nc.vector.tensor_mul(a[:], tmp[:], ps_b[:])

                # mm2: out[t, h], 8 m-tiles per super-chunk. k-outer/
                # n-inner keeps 4 PSUM banks accumulating; the very last
                # m-tile flips to n-outer so each bank finishes early and
                # its copy + store overlap the remaining matmuls. The 4
                # n-block copies pack into ONE [128, 2048] SBUF tile so
                # the store is a single contiguous 512KB DMA.
                for m in range(NT):
                    i, mc = divmod(m, NT // NCI)
                    last = (S == NSC - 1) and (m == NT - 1)
                    osb = outp.tile([128, H], bf16, tag="osb")
                    r0 = (S * NCI + i) * CHUNK + mc * 128
                    if last:
                        # n-outer, and each n-slice stores right after its
                        # copy (sync queue): when the final matmul retires
                        # only one 64KB store remains -> short tail. The
                        # final 512-col block runs as two 256-col chains
                        # so the very last copy/store is half-size. Each
                        # piece gets its OWN psum tile: sharing one tile
                        # between the two 256 pieces made Tile serialize
                        # piece 2's chain behind piece 1's copy.
                        pieces = [(n * 512, 512) for n in range(NHO - 1)]
                        pieces += [(1536, 256), (1792, 256)]
                        for c0, cw in pieces:
                            pp = psp.tile(
                                [128, cw], f32, tag="ps", name=f"po_{S}_{m}_{c0}"
                            )
                            for k in range(NF):
                                nc.tensor.matmul(
                                    pp[:],
                                    act_t[i][k][:, mc * 128 : (mc + 1) * 128],
                                    w2_t[k][:, c0 : c0 + cw],
                                    start=(k == 0),
                                    stop=(k == NF - 1),
                                )
                            nc.scalar.copy(osb[:, c0 : c0 + cw], pp[:])
                            nc.sync.dma_start(
                                out=out_d[r0 : r0 + 128, c0 : c0 + cw],
                                in_=osb[:, c0 : c0 + cw],
                            )
                    else:
                        po = [
                            psp.tile(
                                [128, 512], f32, tag="ps", name=f"po_{S}_{m}_{n}"
                            )
                            for n in range(NHO)
                        ]
                        for k in range(NF):
                            lhsT = act_t[i][k][:, mc * 128 : (mc + 1) * 128]
                            for n in range(NHO):
                                nc.tensor.matmul(
                                    po[n][:],
                                    lhsT,
                                    w2_t[k][:, n * 512 : (n + 1) * 512],
                                    start=(k == 0),
                                    stop=(k == NF - 1),
                                )
                        for n in range(NHO):
                            nc.scalar.copy(
                                osb[:, n * 512 : (n + 1) * 512], po[n][:]
                            )
                        nc.scalar.dma_start(out=out_d[r0 : r0 + 128, :], in_=osb[:])
    if not nc.is_finalized():
        nc.finalize()  # Bacc.finalize runs the lowering pipeline (sem split, alloc_regs)
    return nc


def _get_nc():
    if "nc" not in _CACHE:
        _CACHE["nc"] = _build()
    return _CACHE["nc"]


def _pack_w1(w1e: np.ndarray) -> np.ndarray:
    """[H, 2816] -> [128, NF*KH*256] DMA-native layout.

    Swiglu pair j = (a_j = cols [128j,128j+128), b_j = cols
    [1408+128j, ...)). Per partition p (row index within a k-tile), the
    packed layout is [pair j][k][a_j|b_j cols], so one pair's whole
    k-range is contiguous per partition.
    """
    a = w1e[:, :F].reshape(KH, 128, NF, 128)   # [k, p, j, c]
    b = w1e[:, F:].reshape(KH, 128, NF, 128)
    pair = np.stack([a, b], axis=3)            # [k, p, j, 2, c]
    # -> [p, j, k, 2*128]
    return pair.transpose(1, 2, 0, 3, 4).reshape(128, NF * KH * 256)


def kernel(permuted_hidden_states, num_tokens_per_expert, w1, w2):
    from concourse.bass_utils import run_bass_kernel_spmd

    x = np.asarray(permuted_hidden_states, dtype=np.float32)
    w1 = np.asarray(w1, dtype=np.float32)
    w2 = np.asarray(w2, dtype=np.float32)
    ntpe = np.asarray(num_tokens_per_expert)
    assert x.shape == (T_TOTAL, H) and w1.shape == (E, H, F2) and w2.shape == (E, F, H)
    # Reference semantics rely on the static equal split.
    assert np.all(ntpe == TPC), f"expected equal {TPC}-token splits, got {ntpe}"

    bf = ml_dtypes.bfloat16
    in_maps = []
    NCH = TPC // CHUNK
    for e in range(E):
        xe = x[e * TPC : (e + 1) * TPC]
        # pack as [p, chunk, k, t] so any k-range of a chunk is one
        # contiguous 2D DMA slice (see xT_d comment in _build).
        xg = (
            xe.reshape(NCH, CHUNK, KH, 128)
            .transpose(3, 0, 2, 1)
            .reshape(128, NCH * KH * CHUNK)
        )
        in_maps.append(
            {
                "xT": np.ascontiguousarray(xg).astype(bf),
                "w1": np.ascontiguousarray(_pack_w1(w1[e])).astype(bf),
                "w2": np.ascontiguousarray(w2[e]).astype(bf),
            }
        )

    nc = _get_nc()
    res = run_bass_kernel_spmd(nc, in_maps, list(range(E)), trace=TRACE)
    LAST["exec_time_ns"] = res.exec_time_ns
    LAST["mean_exec_time_ns"] = res.mean_exec_time_ns
    LAST["profile_json"] = res.profile_json
    out = np.concatenate([res.results[i]["out"] for i in range(E)], axis=0)
    return np.ascontiguousarray(out.astype(np.float32))


# revision 17
# speedup vs baseline: 1.0196x; 1.0002x over previous
"""MoE expert FFN (swiglu) kernel for 8 trn2 NeuronCores.

Expert parallelism: 8 experts, one per core. Each core computes, for its
expert e:
    h   = x_e @ w1_e            # [2048, 2048] @ [2048, 2816]
    act = silu(h[:, :1408]) * h[:, 1408:]
    out = act @ w2_e            # [2048, 1408] @ [1408, 2048]

Tokens arrive pre-sorted by expert with equal counts (2048/expert), so
sharding is a static slice and the gather is a concat. No collectives.

Device-side layout (all bf16 compute, fp32 PSUM accumulation, bf16 out):
  mm1: out[f, t] tiles; lhsT = w1[h,f] 128x128 tiles (stationary),
       rhs = xT[h, t] (moving, N=512) -> inter is [f, t], the layout mm2
       needs, so no on-device transpose anywhere (x is transposed on host).
  swiglu pairs: w1 columns are interleaved on HOST so pair j = cols
       [256j, 256j+256) = [a_j | b_j]; act_j = silu(a)*b via ACT(Silu)
       + DVE mul -> bf16 SBUF.
  mm2: out[t, h] tiles; lhsT = act[f, t] 128-col slices (stationary),
       rhs = w2[f, h] (moving, N=512). PSUM -> SBUF bf16 -> DMA to out.

v6: DMA-native host packing. The v5 trace showed the kernel at the PE
instruction floor (216ns per N=512 matmul) with ~35us of overhead, all
DMA-induced: w1 k-slices were strided reads (512B segments, 5.6KB
stride -> ~25GB/s effective), causing 13us before the first matmul and
12us of PE gaps in the first 65us, plus strided output stores in the
tail. Fixes:
  - w1 is host-packed per swiglu pair as [128 partitions][k][256] so a
    pair's full k-range is ONE contiguous-per-partition DMA (8KB/part).
    Pair 0 is split in 4 granules and pair 1 in 2 so the PE can start
    as soon as the first 256KB lands.
  - output stores one full m-tile row-block [128, 2048] per DMA
    (contiguous 512KB) after packing the 4 n-block PSUM copies into a
    single SBUF tile.
  - queue split: sync HWDGE = all w1 then all w2 (its 4-slot rotation
    gives a 4-transfer prefetch window); the startup-critical x chunks
    0 and 1 are each split across gpsimd SWDGE (first k half) and
    scalar HWDGE (second half) so they land in half the time; scalar
    also carries the output stores (its instruction stream runs silu +
    PSUM copies, so it must carry few DMA configs).
  - 9 warmup matmuls on a zeroed scratch tile fill the pre-first-data
    window so the PE's HAM clock gate (cold = 1.2GHz for the first
    ~3.4us of activity) warms on throwaway work instead of real work.

mm1 is software-staggered over two 512-token chunks (step t runs pair
t-2 of chunk 1, then pair t of chunk 0) per 1024-token super-chunk;
weights are then reused across chunks at half the stream rate, and
running the resident-weight chunk-1 chain first gives every fresh w1
pair a full extra chain (~7us) of DMA slack. mm2 runs per super-chunk
(8 m-tiles); the very last m-tile is n-outer with per-n-block stores
(final block as two 256-col chains, each with its own PSUM tile) so
only a 64KB store remains after the last matmul -> short tail.

Weights stay resident in SBUF (bf16: 88KB + 44KB per partition).
PE-bound: ~456us of matmul per core at 2.4GHz; target is wall ~= that.
"""

import os
import sys

sys.path.insert(0, "/opt/trn_rl_repo")

import numpy as np
import ml_dtypes

E = 8             # experts == cores
T_TOTAL = 16384
H = 2048
F = 1408
F2 = 2 * F        # 2816
TPC = T_TOTAL // E  # 2048 tokens per core
CHUNK = 512
NSC = 2                     # super-chunks
NCI = 2                     # chunks per super-chunk
KH = H // 128               # 16 contraction tiles for mm1
NF = F // 128               # 11 swiglu pairs
NT = (NCI * CHUNK) // 128   # 8 m-tiles per super-chunk in mm2
NHO = H // 512              # 4 output column blocks

# DMA granules per w1 pair: pair j is stored [128][k][256] contiguously;
# G[j] transfers cover its KH k-slices. Pair 0 gates the PE start, so it
# is split fine; later pairs arrive a full step ahead at 1MB each.
W1_GRAN = {0: 4, 1: 2}

_CACHE = {}

# Optional knobs read by test.py (not used by the grading harness).
TRACE = os.environ.get("BASS_TRACE_KERNEL", "0") == "1"
LAST = {}


def _build():
    from concourse import bacc, tile, mybir

    bf16 = mybir.dt.bfloat16
    f32 = mybir.dt.float32
    SILU = mybir.ActivationFunctionType.Silu

    # Bacc (not plain Bass): its lowering pipeline splits multi-sem waits
    # into EventSemaphore pairs — TRN2 allows at most 1 wait per instruction.
    nc = bacc.Bacc()
    # x is host-packed as [p, chunk, k, t] -> [128, NCH*KH*CHUNK]: any k-range
    # of one chunk is a single contiguous 2D DMA slice, so x streams in
    # k-PAIR transfers (half the configs of per-k slices).
    xT_d = nc.declare_dram_parameter(
        "xT", [128, (TPC // CHUNK) * KH * CHUNK], bf16, isOutput=False
    )
    # w1 host-packed per partition as [pair][k][256].
    w1_d = nc.declare_dram_parameter("w1", [128, NF * KH * 256], bf16, isOutput=False)
    w2_d = nc.declare_dram_parameter("w2", [F, H], bf16, isOutput=False)
    # bf16 output (host upcasts): halves store bytes + the kernel-tail
    # drain of the final stores. Adds ~0.3% rounding noise on top of the
    # 0.41% bf16-matmul noise — far inside the 2e-2 gate.
    out_d = nc.declare_dram_parameter("out", [TPC, H], bf16, isOutput=True)

    def x_dram_pair(c, kp):
        c0 = (c * KH + 2 * kp) * CHUNK
        return xT_d[:, c0 : c0 + 2 * CHUNK]

    with tile.TileContext(nc) as tc:
        with (
            tc.tile_pool(name="w1p", bufs=1) as w1p,
            tc.tile_pool(name="w2p", bufs=1) as w2p,
            tc.tile_pool(name="xp", bufs=1) as xp,
            tc.tile_pool(name="actp", bufs=1) as actp,
            tc.tile_pool(name="tmpp", bufs=2) as tmpp,
            tc.tile_pool(name="warmp", bufs=1) as warmp,
            tc.tile_pool(name="outp", bufs=2) as outp,
            tc.tile_pool(name="psp", bufs=8, space="PSUM") as psp,
        ):
            # PE warmup: the HAM clock gate holds the PE at 1.2GHz until
            # ~3.4us of sustained activity. Real data lands ~9-10us in
            # (preamble + first transfers), so spend the wait on matmuls
            # over a zeroed scratch tile; the first real matmuls then run
            # at (or much closer to) 2.4GHz. The scratch PSUM tile shares
            # the "ps" tag rotation, so it simply becomes the first of
            # the 8 rotating bank buffers.
            wsrc = warmp.tile([128, 640], bf16, tag="warm")
            nc.vector.memset(wsrc[:], 0.0)
            wps = psp.tile([128, 512], f32, tag="ps", name="warm_ps")
            for _ in range(9):
                nc.tensor.matmul(
                    wps[:], wsrc[:, 0:128], wsrc[:, 128:640], start=True, stop=True
                )

            # x chunk 0 on gpsimd (SWDGE). Irregular k-split
            # [k0][k1,k2]...[k13,k14][k15]: k0 lands first (PE start),
            # and the k15 single rides the scalar queue instead (arrives
            # early vs at the end of gpsimd's stream), so step 0 never
            # waits on its last contraction slice.
            # i-chunk k -> (tile index, col offset) for this split:
            xmap0 = [(0, 0)]
            for k in range(1, KH - 1):
                xmap0.append(((k + 1) // 2, 0) if k % 2 == 1 else (k // 2, CHUNK))
            xmap0.append((KH // 2, 0))

            def x_tiles_i0(c, tag_prefix, name_prefix):
                """Allocate the irregular x tile set for an even chunk c;
                returns (tiles, dma list as (tile, src))."""
                tiles, dmas = [], []
                base = c * KH * CHUNK
                t = xp.tile([128, CHUNK], bf16, tag=f"{tag_prefix}_s0",
                            name=f"{name_prefix}_s0")
                tiles.append(t)
                dmas.append((t, xT_d[:, base : base + CHUNK]))
                for j in range(1, KH // 2):
                    t = xp.tile([128, 2 * CHUNK], bf16, tag=f"{tag_prefix}_p{j}",
                                name=f"{name_prefix}_p{j}")
                    tiles.append(t)
                    c0 = base + (2 * j - 1) * CHUNK
                    dmas.append((t, xT_d[:, c0 : c0 + 2 * CHUNK]))
                t = xp.tile([128, CHUNK], bf16, tag=f"{tag_prefix}_s8",
                            name=f"{name_prefix}_s8")
                tiles.append(t)
                c0 = base + (KH - 1) * CHUNK
                dmas.append((t, xT_d[:, c0 : c0 + CHUNK]))
                return tiles, dmas

            # x chunk 0 is THE startup-critical transfer set (2.2MB the
            # first chain consumes over ~7us): split it across the gpsimd
            # SWDGE queue (k0-6) and the scalar HWDGE queue (k7-15) so it
            # arrives in roughly half the time of a single queue.
            x0_t, x0_dmas = x_tiles_i0(0, "x_0", "x0")
            for t, src in x0_dmas[:4]:
                nc.gpsimd.dma_start(out=t[:], in_=src)
            for t, src in x0_dmas[4:]:
                nc.scalar.dma_start(out=t[:], in_=src)

            # w1 on sync, in consumption order. Pair j's granule g is the
            # contiguous slice [(j*KH + g*KG)*256, +KG*256) per partition.
            # tiles: w1_t[j][g] of [128, KG*256]; matmul slices columns.
            w1_t = []
            for j in range(NF):
                ng = W1_GRAN.get(j, 1)
                kg = KH // ng
                tiles = []
                for g in range(ng):
                    t = w1p.tile([128, kg * 256], bf16, tag=f"w1_{j}_{g}")
                    tiles.append(t)
                    c0 = (j * KH + g * kg) * 256
                    nc.sync.dma_start(out=t[:], in_=w1_d[:, c0 : c0 + kg * 256])
                w1_t.append(tiles)

            def w1_slice(j, k, half):
                ng = W1_GRAN.get(j, 1)
                kg = KH // ng
                t = w1_t[j][k // kg]
                c = (k % kg) * 256 + half * 128
                return t[:, c : c + 128]

            # x chunk 1 split across gpsimd (k0-7) + scalar (k8-15), each
            # behind its x0 half: first used at step 2 (~22us), and the
            # chunk-1 chain now runs first within its step.
            x1_t = []
            for kp in range(KH // 2):
                t = xp.tile([128, 2 * CHUNK], bf16, tag=f"x_1_{kp}", name=f"x1_{kp}")
                x1_t.append(t)
                eng = nc.gpsimd if kp < KH // 4 else nc.scalar
                eng.dma_start(out=t[:], in_=x_dram_pair(1, kp))

            # Resident w2: 11 tiles [128, 2048], each a fully contiguous
            # 512KB DRAM slice; on sync behind w1 (needed ~150us in).
            w2_t = []
            for k in range(NF):
                t = w2p.tile([128, H], bf16, tag=f"w2_{k}")
                w2_t.append(t)
                nc.sync.dma_start(out=t[:], in_=w2_d[k * 128 : (k + 1) * 128, :])

            for S in range(NSC):
                if S == 0:
                    x_t = [x0_t, x1_t]
                else:
                    # Chunks 2+3 reuse chunk 0+1's SBUF slots (released at
                    # the end of super-chunk 0's mm1); the gpsimd queue is
                    # idle by then so the waits cost nothing.
                    c2_t, c2_dmas = x_tiles_i0(S * NCI, "x_0", f"x_{S}_0")
                    for t, src in c2_dmas:
                        nc.gpsimd.dma_start(out=t[:], in_=src)
                    x_t = [c2_t, []]
                    for kp in range(KH // 2):
                        t = xp.tile(
                            [128, 2 * CHUNK], bf16, tag=f"x_1_{kp}",
                            name=f"x_{S}_1_{kp}",
                        )
                        x_t[1].append(t)
                        nc.gpsimd.dma_start(
                            out=t[:], in_=x_dram_pair(S * NCI + 1, kp)
                        )

                # mm1 + swiglu, software-staggered: step t runs pair t of
                # chunk 0 and pair t-2 of chunk 1. Chunk 1's weights are
                # always two pairs old (resident), so only (x0, pair0)
                # are on the DMA critical path at startup. (STAG=4 was
                # tried and regressed: single-chain prefix steps consume
                # w1 pairs at double rate, so the weight deadlines move
                # earlier by exactly what x1's deadline gains.)
                STAG = 2
                act_t = [[None] * NF, [None] * NF]  # [chunk][j]
                for t_s in range(NF + STAG):
                    # Chunk-1's chain runs FIRST within a step: its weights
                    # (pair t-2) are already resident, so the fresh pair t
                    # gets an extra chain (~7us) of DMA slack before the
                    # chunk-0 chain needs it.
                    chains = []
                    if t_s >= STAG:
                        chains.append((1, t_s - STAG))
                    if t_s < NF:
                        chains.append((0, t_s))
                    for i, j in chains:
                        ps_a = psp.tile(
                            [128, CHUNK], f32, tag="ps", name=f"ps_{S}_{i}_{j}_a"
                        )
                        ps_b = psp.tile(
                            [128, CHUNK], f32, tag="ps", name=f"ps_{S}_{i}_{j}_b"
                        )
                        for k in range(KH):
                            st, sp = (k == 0), (k == KH - 1)
                            if i == 0:
                                ti, xo = xmap0[k]
                            else:
                                ti, xo = k // 2, (k % 2) * CHUNK
                            xk = x_t[i][ti][:, xo : xo + CHUNK]
                            nc.tensor.matmul(
                                ps_a[:], w1_slice(j, k, 0), xk, start=st, stop=sp
                            )
                            nc.tensor.matmul(
                                ps_b[:], w1_slice(j, k, 1), xk, start=st, stop=sp
                            )
                        tmp = tmpp.tile([128, CHUNK], f32, tag="tmp")
                        nc.scalar.activation(tmp[:], ps_a[:], SILU)
                        a = actp.tile([128, CHUNK], bf16, tag=f"act_{i}_{j}")
                        act_t[i][j] = a
                        nc.vector.tensor_mul(a[:], tmp[:], ps_b[:])

                # mm2: out[t, h], 8 m-tiles per super-chunk. k-outer/
                # n-inner keeps 4 PSUM banks accumulating; the very last
                # m-tile flips to n-outer so each bank finishes early and
                # its copy + store overlap the remaining matmuls. The 4
                # n-block copies pack into ONE [128, 2048] SBUF tile so
                # the store is a single contiguous 512KB DMA.
                for m in range(NT):
                    i, mc = divmod(m, NT // NCI)
                    last = (S == NSC - 1) and (m == NT - 1)
                    osb = outp.tile([128, H], bf16, tag="osb")
                    r0 = (S * NCI + i) * CHUNK + mc * 128
                    if last:
                        # n-outer, and each n-slice stores right after its
                        # copy (sync queue): when the final matmul retires
                        # only one 64KB store remains -> short tail. The
                        # final 512-col block runs as two 256-col chains
                        # so the very last copy/store is half-size. Each
                        # piece gets its OWN psum tile: sharing one tile
                        # between the two 256 pieces made Tile serialize
                        # piece 2's chain behind piece 1's copy.
                        pieces = [(n * 512, 512) for n in range(NHO - 1)]
                        pieces += [(1536, 256), (1792, 256)]
                        for c0, cw in pieces:
                            pp = psp.tile(
                                [128, cw], f32, tag="ps", name=f"po_{S}_{m}_{c0}"
                            )
                            for k in range(NF):
                                nc.tensor.matmul(
                                    pp[:],
                                    act_t[i][k][:, mc * 128 : (mc + 1) * 128],
                                    w2_t[k][:, c0 : c0 + cw],
                                    start=(k == 0),
                                    stop=(k == NF - 1),
                                )
                            nc.scalar.copy(osb[:, c0 : c0 + cw], pp[:])
                            nc.sync.dma_start(
                                out=out_d[r0 : r0 + 128, c0 : c0 + cw],
                                in_=osb[:, c0 : c0 + cw],
                            )
                    else:
                        po = [
                            psp.tile(
                                [128, 512], f32, tag="ps", name=f"po_{S}_{m}_{n}"
                            )
                            for n in range(NHO)
                        ]
                        for k in range(NF):
                            lhsT = act_t[i][k][:, mc * 128 : (mc + 1) * 128]
                            for n in range(NHO):
                                nc.tensor.matmul(
                                    po[n][:],
                                    lhsT,
                                    w2_t[k][:, n * 512 : (n + 1) * 512],
                                    start=(k == 0),
                                    stop=(k == NF - 1),
                                )
                        for n in range(NHO):
                            nc.scalar.copy(
                                osb[:, n * 512 : (n + 1) * 512], po[n][:]
                            )
                        nc.scalar.dma_start(out=out_d[r0 : r0 + 128, :], in_=osb[:])
    if not nc.is_finalized():
        nc.finalize()  # Bacc.finalize runs the lowering pipeline (sem split, alloc_regs)
    return nc


def _get_nc():
    if "nc" not in _CACHE:
        _CACHE["nc"] = _build()
    return _CACHE["nc"]


def _pack_w1(w1e: np.ndarray) -> np.ndarray:
    """[H, 2816] -> [128, NF*KH*256] DMA-native layout.

    Swiglu pair j = (a_j = cols [128j,128j+128), b_j = cols
    [1408+128j, ...)). Per partition p (row index within a k-tile), the
    packed layout is [pair j][k][a_j|b_j cols], so one pair's whole
    k-range is contiguous per partition.
    """
    a = w1e[:, :F].reshape(KH, 128, NF, 128)   # [k, p, j, c]
    b = w1e[:, F:].reshape(KH, 128, NF, 128)
    pair = np.stack([a, b], axis=3)            # [k, p, j, 2, c]
    # -> [p, j, k, 2*128]
    return pair.transpose(1, 2, 0, 3, 4).reshape(128, NF * KH * 256)


def kernel(permuted_hidden_states, num_tokens_per_expert, w1, w2):
    from concourse.bass_utils import run_bass_kernel_spmd

    x = np.asarray(permuted_hidden_states, dtype=np.float32)
    w1 = np.asarray(w1, dtype=np.float32)
    w2 = np.asarray(w2, dtype=np.float32)
    ntpe = np.asarray(num_tokens_per_expert)
    assert x.shape == (T_TOTAL, H) and w1.shape == (E, H, F2) and w2.shape == (E, F, H)
    # Reference semantics rely on the static equal split.
    assert np.all(ntpe == TPC), f"expected equal {TPC}-token splits, got {ntpe}"

    bf = ml_dtypes.bfloat16
    in_maps = []
    NCH = TPC // CHUNK
    for e in range(E):
        xe = x[e * TPC : (e + 1) * TPC]
        # pack as [p, chunk, k, t] so any k-range of a chunk is one
        # contiguous 2D DMA slice (see xT_d comment in _build).
        xg = (
            xe.reshape(NCH, CHUNK, KH, 128)
            .transpose(3, 0, 2, 1)
            .reshape(128, NCH * KH * CHUNK)
        )
        in_maps.append(
            {
                "xT": np.ascontiguousarray(xg).astype(bf),
                "w1": np.ascontiguousarray(_pack_w1(w1[e])).astype(bf),
                "w2": np.ascontiguousarray(w2[e]).astype(bf),
            }
        )

    nc = _get_nc()
    res = run_bass_kernel_spmd(nc, in_maps, list(range(E)), trace=TRACE)
    LAST["exec_time_ns"] = res.exec_time_ns
    LAST["mean_exec_time_ns"] = res.mean_exec_time_ns
    LAST["profile_json"] = res.profile_json
    out = np.concatenate([res.results[i]["out"] for i in range(E)], axis=0)
    return np.ascontiguousarray(out.astype(np.float32))


# revision 18
# speedup vs baseline: 1.0198x; 1.0002x over previous
"""MoE expert FFN (swiglu) kernel for 8 trn2 NeuronCores.

Expert parallelism: 8 experts, one per core. Each core computes, for its
expert e:
    h   = x_e @ w1_e            # [2048, 2048] @ [2048, 2816]
    act = silu(h[:, :1408]) * h[:, 1408:]
    out = act @ w2_e            # [2048, 1408] @ [1408, 2048]

Tokens arrive pre-sorted by expert with equal counts (2048/expert), so
sharding is a static slice and the gather is a concat. No collectives.

Device-side layout (all bf16 compute, fp32 PSUM accumulation, bf16 out):
  mm1: out[f, t] tiles; lhsT = w1[h,f] 128x128 tiles (stationary),
       rhs = xT[h, t] (moving, N=512) -> inter is [f, t], the layout mm2
       needs, so no on-device transpose anywhere (x is transposed on host).
  swiglu pairs: w1 columns are interleaved on HOST so pair j = cols
       [256j, 256j+256) = [a_j | b_j]; act_j = silu(a)*b via ACT(Silu)
       + DVE mul -> bf16 SBUF.
  mm2: out[t, h] tiles; lhsT = act[f, t] 128-col slices (stationary),
       rhs = w2[f, h] (moving, N=512). PSUM -> SBUF bf16 -> DMA to out.

v6: DMA-native host packing. The v5 trace showed the kernel at the PE
instruction floor (216ns per N=512 matmul) with ~35us of overhead, all
DMA-induced: w1 k-slices were strided reads (512B segments, 5.6KB
stride -> ~25GB/s effective), causing 13us before the first matmul and
12us of PE gaps in the first 65us, plus strided output stores in the
tail. Fixes:
  - w1 is host-packed per swiglu pair as [128 partitions][k][256] so a
    pair's full k-range is ONE contiguous-per-partition DMA (8KB/part).
    Pair 0 is split in 4 granules and pair 1 in 2 so the PE can start
    as soon as the first 256KB lands.
  - output stores one full m-tile row-block [128, 2048] per DMA
    (contiguous 512KB) after packing the 4 n-block PSUM copies into a
    single SBUF tile.
  - queue split: sync HWDGE = all w1 then all w2 (its 4-slot rotation
    gives a 4-transfer prefetch window); the startup-critical x chunks
    0 and 1 are each split across gpsimd SWDGE (first k half) and
    scalar HWDGE (second half) so they land in half the time; scalar
    also carries the output stores (its instruction stream runs silu +
    PSUM copies, so it must carry few DMA configs).
  - 9 warmup matmuls on a zeroed scratch tile fill the pre-first-data
    window so the PE's HAM clock gate (cold = 1.2GHz for the first
    ~3.4us of activity) warms on throwaway work instead of real work.

mm1 is software-staggered over two 512-token chunks (step t runs pair
t-2 of chunk 1, then pair t of chunk 0) per 1024-token super-chunk;
weights are then reused across chunks at half the stream rate, and
running the resident-weight chunk-1 chain first gives every fresh w1
pair a full extra chain (~7us) of DMA slack. mm2 runs per super-chunk
(8 m-tiles); the very last m-tile is n-outer with per-n-block stores
(final block as two 256-col chains, each with its own PSUM tile) so
only a 64KB store remains after the last matmul -> short tail.

Weights stay resident in SBUF (bf16: 88KB + 44KB per partition).
PE-bound: ~456us of matmul per core at 2.4GHz; target is wall ~= that.
"""

import os
import sys

sys.path.insert(0, "/opt/trn_rl_repo")

import numpy as np
import ml_dtypes

E = 8             # experts == cores
T_TOTAL = 16384
H = 2048
F = 1408
F2 = 2 * F        # 2816
TPC = T_TOTAL // E  # 2048 tokens per core
CHUNK = 512
NSC = 2                     # super-chunks
NCI = 2                     # chunks per super-chunk
KH = H // 128               # 16 contraction tiles for mm1
NF = F // 128               # 11 swiglu pairs
NT = (NCI * CHUNK) // 128   # 8 m-tiles per super-chunk in mm2
NHO = H // 512              # 4 output column blocks

# DMA granules per w1 pair: pair j is stored [128][k][256] contiguously;
# G[j] transfers cover its KH k-slices. Pair 0 gates the PE start, so it
# is split fine; later pairs arrive a full step ahead at 1MB each.
W1_GRAN = {0: 4, 1: 2}

_CACHE = {}

# Optional knobs read by test.py (not used by the grading harness).
TRACE = os.environ.get("BASS_TRACE_KERNEL", "0") == "1"
LAST = {}


def _build():
    from concourse import bacc, tile, mybir

    bf16 = mybir.dt.bfloat16
    f32 = mybir.dt.float32
    SILU = mybir.ActivationFunctionType.Silu

    # Bacc (not plain Bass): its lowering pipeline splits multi-sem waits
    # into EventSemaphore pairs — TRN2 allows at most 1 wait per instruction.
    nc = bacc.Bacc()
    # x is host-packed as [p, chunk, k, t] -> [128, NCH*KH*CHUNK]: any k-range
    # of one chunk is a single contiguous 2D DMA slice, so x streams in
    # k-PAIR transfers (half the configs of per-k slices).
    xT_d = nc.declare_dram_parameter(
        "xT", [128, (TPC // CHUNK) * KH * CHUNK], bf16, isOutput=False
    )
    # w1 host-packed per partition as [pair][k][256].
    w1_d = nc.declare_dram_parameter("w1", [128, NF * KH * 256], bf16, isOutput=False)
    w2_d = nc.declare_dram_parameter("w2", [F, H], bf16, isOutput=False)
    # bf16 output (host upcasts): halves store bytes + the kernel-tail
    # drain of the final stores. Adds ~0.3% rounding noise on top of the
    # 0.41% bf16-matmul noise — far inside the 2e-2 gate.
    out_d = nc.declare_dram_parameter("out", [TPC, H], bf16, isOutput=True)

    def x_dram_pair(c, kp):
        c0 = (c * KH + 2 * kp) * CHUNK
        return xT_d[:, c0 : c0 + 2 * CHUNK]

    with tile.TileContext(nc) as tc:
        with (
            tc.tile_pool(name="w1p", bufs=1) as w1p,
            tc.tile_pool(name="w2p", bufs=1) as w2p,
            tc.tile_pool(name="xp", bufs=1) as xp,
            tc.tile_pool(name="actp", bufs=1) as actp,
            tc.tile_pool(name="tmpp", bufs=2) as tmpp,
            tc.tile_pool(name="warmp", bufs=1) as warmp,
            tc.tile_pool(name="outp", bufs=2) as outp,
            tc.tile_pool(name="psp", bufs=8, space="PSUM") as psp,
        ):
            # PE warmup: the HAM clock gate holds the PE at 1.2GHz until
            # ~3.4us of sustained activity. Real data lands ~9-10us in
            # (preamble + first transfers), so spend the wait on matmuls
            # over a zeroed scratch tile; the first real matmuls then run
            # at (or much closer to) 2.4GHz. The scratch PSUM tile shares
            # the "ps" tag rotation, so it simply becomes the first of
            # the 8 rotating bank buffers.
            wsrc = warmp.tile([128, 640], bf16, tag="warm")
            nc.vector.memset(wsrc[:], 0.0)
            wps = psp.tile([128, 512], f32, tag="ps", name="warm_ps")
            for _ in range(9):
                nc.tensor.matmul(
                    wps[:], wsrc[:, 0:128], wsrc[:, 128:640], start=True, stop=True
                )

            # x chunk 0 on gpsimd (SWDGE). Irregular k-split
            # [k0][k1,k2]...[k13,k14][k15]: k0 lands first (PE start),
            # and the k15 single rides the scalar queue instead (arrives
            # early vs at the end of gpsimd's stream), so step 0 never
            # waits on its last contraction slice.
            # i-chunk k -> (tile index, col offset) for this split:
            xmap0 = [(0, 0)]
            for k in range(1, KH - 1):
                xmap0.append(((k + 1) // 2, 0) if k % 2 == 1 else (k // 2, CHUNK))
            xmap0.append((KH // 2, 0))

            def x_tiles_i0(c, tag_prefix, name_prefix):
                """Allocate the irregular x tile set for an even chunk c;
                returns (tiles, dma list as (tile, src))."""
                tiles, dmas = [], []
                base = c * KH * CHUNK
                t = xp.tile([128, CHUNK], bf16, tag=f"{tag_prefix}_s0",
                            name=f"{name_prefix}_s0")
                tiles.append(t)
                dmas.append((t, xT_d[:, base : base + CHUNK]))
                for j in range(1, KH // 2):
                    t = xp.tile([128, 2 * CHUNK], bf16, tag=f"{tag_prefix}_p{j}",
                                name=f"{name_prefix}_p{j}")
                    tiles.append(t)
                    c0 = base + (2 * j - 1) * CHUNK
                    dmas.append((t, xT_d[:, c0 : c0 + 2 * CHUNK]))
                t = xp.tile([128, CHUNK], bf16, tag=f"{tag_prefix}_s8",
                            name=f"{name_prefix}_s8")
                tiles.append(t)
                c0 = base + (KH - 1) * CHUNK
                dmas.append((t, xT_d[:, c0 : c0 + CHUNK]))
                return tiles, dmas

            # x chunk 0 is THE startup-critical transfer set (2.2MB the
            # first chain consumes over ~7us): split it across the gpsimd
            # SWDGE queue (k0-6) and the scalar HWDGE queue (k7-15) so it
            # arrives in roughly half the time of a single queue.
            x0_t, x0_dmas = x_tiles_i0(0, "x_0", "x0")
            for t, src in x0_dmas[:4]:
                nc.gpsimd.dma_start(out=t[:], in_=src)
            for t, src in x0_dmas[4:]:
                nc.scalar.dma_start(out=t[:], in_=src)

            # w1 on sync, in consumption order. Pair j's granule g is the
            # contiguous slice [(j*KH + g*KG)*256, +KG*256) per partition.
            # tiles: w1_t[j][g] of [128, KG*256]; matmul slices columns.
            w1_t = []
            for j in range(NF):
                ng = W1_GRAN.get(j, 1)
                kg = KH // ng
                tiles = []
                for g in range(ng):
                    t = w1p.tile([128, kg * 256], bf16, tag=f"w1_{j}_{g}")
                    tiles.append(t)
                    c0 = (j * KH + g * kg) * 256
                    nc.sync.dma_start(out=t[:], in_=w1_d[:, c0 : c0 + kg * 256])
                w1_t.append(tiles)

            def w1_slice(j, k, half):
                ng = W1_GRAN.get(j, 1)
                kg = KH // ng
                t = w1_t[j][k // kg]
                c = (k % kg) * 256 + half * 128
                return t[:, c : c + 128]

            # x chunk 1 split across gpsimd (k0-7) + scalar (k8-15), each
            # behind its x0 half: first used at step 2 (~22us), and the
            # chunk-1 chain now runs first within its step.
            x1_t = []
            for kp in range(KH // 2):
                t = xp.tile([128, 2 * CHUNK], bf16, tag=f"x_1_{kp}", name=f"x1_{kp}")
                x1_t.append(t)
                eng = nc.gpsimd if kp < KH // 4 else nc.scalar
                eng.dma_start(out=t[:], in_=x_dram_pair(1, kp))

            # Resident w2: 11 tiles [128, 2048], each a fully contiguous
            # 512KB DRAM slice; on sync behind w1 (needed ~150us in).
            w2_t = []
            for k in range(NF):
                t = w2p.tile([128, H], bf16, tag=f"w2_{k}")
                w2_t.append(t)
                nc.sync.dma_start(out=t[:], in_=w2_d[k * 128 : (k + 1) * 128, :])

            for S in range(NSC):
                if S == 0:
                    x_t = [x0_t, x1_t]
                else:
                    # Chunks 2+3 reuse chunk 0+1's SBUF slots (released at
                    # the end of super-chunk 0's mm1); the gpsimd queue is
                    # idle by then so the waits cost nothing.
                    c2_t, c2_dmas = x_tiles_i0(S * NCI, "x_0", f"x_{S}_0")
                    for t, src in c2_dmas:
                        nc.gpsimd.dma_start(out=t[:], in_=src)
                    x_t = [c2_t, []]
                    for kp in range(KH // 2):
                        t = xp.tile(
                            [128, 2 * CHUNK], bf16, tag=f"x_1_{kp}",
                            name=f"x_{S}_1_{kp}",
                        )
                        x_t[1].append(t)
                        nc.gpsimd.dma_start(
                            out=t[:], in_=x_dram_pair(S * NCI + 1, kp)
                        )

                # mm1 + swiglu, software-staggered: step t runs pair t of
                # chunk 0 and pair t-2 of chunk 1. Chunk 1's weights are
                # always two pairs old (resident), so only (x0, pair0)
                # are on the DMA critical path at startup. (STAG=4 was
                # tried and regressed: single-chain prefix steps consume
                # w1 pairs at double rate, so the weight deadlines move
                # earlier by exactly what x1's deadline gains.)
                STAG = 2
                act_t = [[None] * NF, [None] * NF]  # [chunk][j]
                for t_s in range(NF + STAG):
                    # Chunk-1's chain runs FIRST within a step: its weights
                    # (pair t-2) are already resident, so the fresh pair t
                    # gets an extra chain (~7us) of DMA slack before the
                    # chunk-0 chain needs it.
                    chains = []
                    if t_s >= STAG:
                        chains.append((1, t_s - STAG))
                    if t_s < NF:
                        chains.append((0, t_s))
                    for ci, (i, j) in enumerate(chains):
                        ps_a = psp.tile(
                            [128, CHUNK], f32, tag="ps", name=f"ps_{S}_{i}_{j}_a"
                        )
                        ps_b = psp.tile(
                            [128, CHUNK], f32, tag="ps", name=f"ps_{S}_{i}_{j}_b"
                        )
                        for k in range(KH):
                            st, sp = (k == 0), (k == KH - 1)
                            if i == 0:
                                ti, xo = xmap0[k]
                            else:
                                ti, xo = k // 2, (k % 2) * CHUNK
                            xk = x_t[i][ti][:, xo : xo + CHUNK]
                            nc.tensor.matmul(
                                ps_a[:], w1_slice(j, k, 0), xk, start=st, stop=sp
                            )
                            nc.tensor.matmul(
                                ps_b[:], w1_slice(j, k, 1), xk, start=st, stop=sp
                            )
                            # Sparse HAM keep-alive during the startup DMA
                            # dribble (first 3 steps of super-chunk 0):
                            # the PE queue is in-order, so a dependency-free
                            # scratch matmul woven between real chain MMs
                            # runs exactly when the real stream stalls on a
                            # DMA wait. One N=128 MM (~107ns worst-case
                            # cost) every 4 k-slices keeps every 3.4us HAM
                            # activity window non-idle, preventing the
                            # re-throttle to 1.2GHz that otherwise follows
                            # a fully-idle window (~2-8us loss per core).
                            if S == 0 and t_s <= 2 and ci == 0 and k % 4 == 3:
                                nc.tensor.matmul(
                                    wps[:, 0:128],
                                    wsrc[:, 0:128],
                                    wsrc[:, 128:256],
                                    start=True,
                                    stop=True,
                                )
                        tmp = tmpp.tile([128, CHUNK], f32, tag="tmp")
                        nc.scalar.activation(tmp[:], ps_a[:], SILU)
                        a = actp.tile([128, CHUNK], bf16, tag=f"act_{i}_{j}")
                        act_t[i][j] = a
                        nc.vector.tensor_mul(a[:], tmp[:], ps_b[:])

                # mm2: out[t, h], 8 m-tiles per super-chunk. k-outer/
                # n-inner keeps 4 PSUM banks accumulating; the very last
                # m-tile flips to n-outer so each bank finishes early and
                # its copy + store overlap the remaining matmuls. The 4
                # n-block copies pack into ONE [128, 2048] SBUF tile so
                # the store is a single contiguous 512KB DMA.
                for m in range(NT):
                    i, mc = divmod(m, NT // NCI)
                    last = (S == NSC - 1) and (m == NT - 1)
                    osb = outp.tile([128, H], bf16, tag="osb")
                    r0 = (S * NCI + i) * CHUNK + mc * 128
                    if last:
                        # n-outer, and each n-slice stores right after its
                        # copy (sync queue): when the final matmul retires
                        # only one 64KB store remains -> short tail. The
                        # final 512-col block runs as two 256-col chains
                        # so the very last copy/store is half-size. Each
                        # piece gets its OWN psum tile: sharing one tile
                        # between the two 256 pieces made Tile serialize
                        # piece 2's chain behind piece 1's copy.
                        pieces = [(n * 512, 512) for n in range(NHO - 1)]
                        pieces += [(1536, 256), (1792, 256)]
                        for c0, cw in pieces:
                            pp = psp.tile(
                                [128, cw], f32, tag="ps", name=f"po_{S}_{m}_{c0}"
                            )
                            for k in range(NF):
                                nc.tensor.matmul(
                                    pp[:],
                                    act_t[i][k][:, mc * 128 : (mc + 1) * 128],
                                    w2_t[k][:, c0 : c0 + cw],
                                    start=(k == 0),
                                    stop=(k == NF - 1),
                                )
                            nc.scalar.copy(osb[:, c0 : c0 + cw], pp[:])
                            nc.sync.dma_start(
                                out=out_d[r0 : r0 + 128, c0 : c0 + cw],
                                in_=osb[:, c0 : c0 + cw],
                            )
                    else:
                        po = [
                            psp.tile(
                                [128, 512], f32, tag="ps", name=f"po_{S}_{m}_{n}"
                            )
                            for n in range(NHO)
                        ]
                        for k in range(NF):
                            lhsT = act_t[i][k][:, mc * 128 : (mc + 1) * 128]
                            for n in range(NHO):
                                nc.tensor.matmul(
                                    po[n][:],
                                    lhsT,
                                    w2_t[k][:, n * 512 : (n + 1) * 512],
                                    start=(k == 0),
                                    stop=(k == NF - 1),
                                )
                        for n in range(NHO):
                            nc.scalar.copy(
                                osb[:, n * 512 : (n + 1) * 512], po[n][:]
                            )
                        nc.scalar.dma_start(out=out_d[r0 : r0 + 128, :], in_=osb[:])
    if not nc.is_finalized():
        nc.finalize()  # Bacc.finalize runs the lowering pipeline (sem split, alloc_regs)
    return nc


def _get_nc():
    if "nc" not in _CACHE:
        _CACHE["nc"] = _build()
    return _CACHE["nc"]


def _pack_w1(w1e: np.ndarray) -> np.ndarray:
    """[H, 2816] -> [128, NF*KH*256] DMA-native layout.

    Swiglu pair j = (a_j = cols [128j,128j+128), b_j = cols
    [1408+128j, ...)). Per partition p (row index within a k-tile), the
    packed layout is [pair j][k][a_j|b_j cols], so one pair's whole
    k-range is contiguous per partition.
    """
    a = w1e[:, :F].reshape(KH, 128, NF, 128)   # [k, p, j, c]
    b = w1e[:, F:].reshape(KH, 128, NF, 128)
    pair = np.stack([a, b], axis=3)            # [k, p, j, 2, c]
    # -> [p, j, k, 2*128]
    return pair.transpose(1, 2, 0, 3, 4).reshape(128, NF * KH * 256)


def kernel(permuted_hidden_states, num_tokens_per_expert, w1, w2):
    from concourse.bass_utils import run_bass_kernel_spmd

    x = np.asarray(permuted_hidden_states, dtype=np.float32)
    w1 = np.asarray(w1, dtype=np.float32)
    w2 = np.asarray(w2, dtype=np.float32)
    ntpe = np.asarray(num_tokens_per_expert)
    assert x.shape == (T_TOTAL, H) and w1.shape == (E, H, F2) and w2.shape == (E, F, H)
    # Reference semantics rely on the static equal split.
    assert np.all(ntpe == TPC), f"expected equal {TPC}-token splits, got {ntpe}"

    bf = ml_dtypes.bfloat16
    in_maps = []
    NCH = TPC // CHUNK
    for e in range(E):
        xe = x[e * TPC : (e + 1) * TPC]
        # pack as [p, chunk, k, t] so any k-range of a chunk is one
        # contiguous 2D DMA slice (see xT_d comment in _build).
        xg = (
            xe.reshape(NCH, CHUNK, KH, 128)
            .transpose(3, 0, 2, 1)
            .reshape(128, NCH * KH * CHUNK)
        )
        in_maps.append(
            {
                "xT": np.ascontiguousarray(xg).astype(bf),
                "w1": np.ascontiguousarray(_pack_w1(w1[e])).astype(bf),
                "w2": np.ascontiguousarray(w2[e]).astype(bf),
            }
        )

    nc = _get_nc()
    res = run_bass_kernel_spmd(nc, in_maps, list(range(E)), trace=TRACE)
    LAST["exec_time_ns"] = res.exec_time_ns
    LAST["mean_exec_time_ns"] = res.mean_exec_time_ns
    LAST["profile_json"] = res.profile_json
    out = np.concatenate([res.results[i]["out"] for i in range(E)], axis=0)
    return np.ascontiguousarray(out.astype(np.float32))
